# revision 13
# baseline (speedup 1.0000x reference)
"""Trainium2 Bass kernel for nn_Encoder_3796751090357 (GNN message passing).

Reference computation (see reference.py):
    x1   = feat   @ W1                      [N, 64]
    z    = S @ x1        (S = sparse adj)   [N, 64]   -> output "hidden_emb"
    emb  = relu(z)                                    -> output "emb"
    x2   = feat_a @ W1
    z_a  = S @ x2 ; emb_a = relu(z_a)
    g    = sigmoid(l2norm((S @ emb)   / rowsum(S)))
    g_a  = sigmoid(l2norm((S @ emb_a) / rowsum(S)))
    ret   = [bilin(emb, g),   bilin(emb_a, g)]        -> output [N, 2]
    ret_a = [bilin(emb_a, g_a), bilin(emb, g_a)]      -> output [N, 2]
    (the reference's `h` tensor is computed but unused -> skipped here)

Sharding: nodes (destination rows) are sharded across the 8 cores; edges are
partitioned by destination row.  Each core computes x1/x2 for its node shard;
an AllGather materializes the full [100352, 128] gather table in every core's
DRAM; each core then runs the SpMMs for its destination shard as one-hot
matmuls: for each 128-row destination block, edges are packed into chunks of
128; each chunk contributes onehot[e, r] = val_e * (lrow_e == r) as PE
weights against the dma_gather'ed source rows as the moving operand,
accumulating the block in PSUM.

The per-edge source rows are fetched with the GPSIMD dma_gather custom op
(mlp ucode library; int16 indices), so the table is split into 4 ranges of
2^15 rows; each destination block issues one gather per range.  Edge slots
are ordered (range, chunk, partition) to match dma_gather's output layout
(index k -> partition k%128, column k//128).
"""

from contextlib import ExitStack

import numpy as np
import ml_dtypes

import concourse.bacc as bacc
import concourse.bass as bass
import concourse.mybir as mybir
import concourse.tile as tile
from concourse.library_config import mlp as _mlp_lib
from concourse.tile import add_dep_helper
from concourse.masks import make_identity  # noqa: F401  (kept for reference)

F32 = mybir.dt.float32
BF16 = mybir.dt.bfloat16
I16 = mybir.dt.int16

NCORES = 8
BLK = 128
KDIM = 512  # IN_F
F = 64      # OUT_F
EPS = 1e-12
RANGE_BITS = 15
RANGE_ROWS = 1 << RANGE_BITS

# gather-table dtypes: phase 1 (x1|x2 -> z) and phase 2 (emb|emb_a -> vsum)
TAB1_BF16 = False
TAB2_BF16 = False


class Cfg:
    def __init__(self, n_nodes, ct_r, tab1_bf16, tab2_bf16):
        self.N = n_nodes
        assert n_nodes % NCORES == 0
        self.Nc = n_nodes // NCORES            # real rows per core
        self.NB = -(-self.Nc // BLK)           # dest blocks per core
        self.NPAD = self.NB * BLK              # padded rows per core
        self.NFULL = self.NPAD * NCORES        # padded rows, all cores
        self.CT_R = list(ct_r)                 # chunks per block, per range
        self.CT = sum(ct_r)                    # total chunks per block
        self.NR = len(ct_r)
        self.tab1_bf16 = tab1_bf16
        self.tab2_bf16 = tab2_bf16


def _npdt(bf16):
    return ml_dtypes.bfloat16 if bf16 else np.float32


# ----------------------------------------------------------------- host side

def preprocess(inputs, tab1_bf16=TAB1_BF16, tab2_bf16=TAB2_BF16):
    """Sort/pad edges, pre-transpose features, build per-core input maps."""
    feat = np.ascontiguousarray(np.asarray(inputs["feat"], dtype=np.float32))
    feat_a = np.ascontiguousarray(np.asarray(inputs["feat_a"], dtype=np.float32))
    vals = np.asarray(inputs["adj_vals"], dtype=np.float32)
    rows = np.asarray(inputs["adj_rows"]).astype(np.int64)
    cols = np.asarray(inputs["adj_cols"]).astype(np.int64)
    w1 = np.ascontiguousarray(np.asarray(inputs["weight1"], dtype=np.float32))
    disc_w = np.asarray(inputs["disc_w"], dtype=np.float32)
    disc_b = np.float32(np.asarray(inputs["disc_b"]))

    n_nodes = feat.shape[0]
    assert n_nodes % NCORES == 0
    nc_rows = n_nodes // NCORES
    nb = -(-nc_rows // BLK)
    npad = nb * BLK
    nfull = npad * NCORES
    n_ranges = -(-nfull // RANGE_ROWS)

    # padded source-row id in the allgathered table + its range
    pid = (cols // nc_rows) * npad + cols % nc_rows
    rng_id = pid >> RANGE_BITS

    shard = rows // nc_rows
    lrow_core = rows - shard * nc_rows
    pblock = shard * nb + lrow_core // BLK        # global dest block id
    lr_all = (lrow_core % BLK).astype(np.float32)

    # order edges by (dest block, source range)
    key = pblock * n_ranges + rng_id
    order = np.argsort(key, kind="stable")
    key_s = key[order]
    pid_s = pid[order]
    vals_s = vals[order]
    lr_s = lr_all[order]

    n_groups = NCORES * nb * n_ranges
    counts = np.bincount(key_s, minlength=n_groups)
    cnt_br = counts.reshape(NCORES * nb, n_ranges)
    ct_r = [int(-(-cnt_br[:, r].max() // BLK)) for r in range(n_ranges)]
    ct_r = [max(c, 1) for c in ct_r]
    cfg = Cfg(n_nodes, ct_r, tab1_bf16, tab2_bf16)
    ct_tot = cfg.CT
    off_c = np.concatenate([[0], np.cumsum(ct_r)])  # chunk offset per range

    starts = np.zeros(n_groups, dtype=np.int64)
    np.cumsum(counts[:-1], out=starts[1:])
    rank = np.arange(len(key_s), dtype=np.int64) - starts[key_s]

    g_rng = key_s % n_ranges
    g_blk = key_s // n_ranges          # global block id (core*nb + b)
    ci = g_blk // nb
    bi = g_blk % nb

    # int16 gather indices: wrapped [16, S] layout, k -> (k%16, k//16);
    # per-range blocks concatenated along S. (S units of 16 idxs.)
    s_r = [c * BLK // 16 for c in ct_r]
    off_s = np.concatenate([[0], np.cumsum(s_r)])
    s_tot = int(off_s[-1])
    idx16 = np.zeros((NCORES, nb, 16, s_tot), dtype=np.int16)
    idx16[ci, bi, rank % 16, off_s[g_rng] + rank // 16] = \
        (pid_s & (RANGE_ROWS - 1)).astype(np.int16)
    idx16 = np.ascontiguousarray(np.tile(idx16, (1, 1, 8, 1)))

    # one-hot operands: slot (p, chunk) with chunk = off_c[range] + rank//128
    lrowf = np.zeros((NCORES, nb, BLK, ct_tot), dtype=np.float32)
    valsf = np.zeros((NCORES, nb, BLK, ct_tot), dtype=np.float32)
    cslot = off_c[g_rng] + rank // BLK
    lrowf[ci, bi, rank % BLK, cslot] = lr_s
    valsf[ci, bi, rank % BLK, cslot] = vals_s

    # pre-transposed feature shards [KDIM, NPAD]
    fT = np.ascontiguousarray(feat.T)
    faT = np.ascontiguousarray(feat_a.T)
    k = feat.shape[1]

    wt2 = np.ascontiguousarray(
        np.concatenate([disc_w.T, disc_w.T], axis=0).astype(np.float32))
    bcol = np.full((BLK, 1), disc_b, dtype=np.float32)
    iota = np.broadcast_to(np.arange(BLK, dtype=np.float32)[None, :],
                           (BLK, BLK)).copy()
    ident = np.eye(BLK, dtype=np.float32)

    d1 = _npdt(tab1_bf16)
    d2 = _npdt(tab2_bf16)
    lrow1 = lrowf.astype(d1)
    vals1 = valsf.astype(d1)
    lrow2 = lrowf.astype(d2)
    vals2 = valsf.astype(d2)

    in_maps = []
    for c in range(NCORES):
        sl = slice(c * nc_rows, (c + 1) * nc_rows)
        ft = np.zeros((k, npad), dtype=np.float32)
        ft[:, :nc_rows] = fT[:, sl]
        fat = np.zeros((k, npad), dtype=np.float32)
        fat[:, :nc_rows] = faT[:, sl]
        in_maps.append({
            "featT": ft,
            "feataT": fat,
            "idx16": idx16[c],
            "lrow1": lrow1[c],
            "vals1": vals1[c],
            "lrow2": lrow2[c],
            "vals2": vals2[c],
            "w1": w1,
            "wt2": wt2,
            "bcol": bcol,
            "iota_in": iota,
            "ident_in": ident,
        })
    return cfg, in_maps


# --------------------------------------------------------------- device side

def build_program(cfg, kdim=KDIM, phases=(1, 1, 1), p2lvl=3):
    NB, CT, NPAD, NFULL = cfg.NB, cfg.CT, cfg.NPAD, cfg.NFULL
    CT_R, NR = cfg.CT_R, cfg.NR
    KC = kdim // BLK  # contraction chunks for the dense matmul
    DT1 = BF16 if cfg.tab1_bf16 else F32
    DT2 = BF16 if cfg.tab2_bf16 else F32
    S_R = [c * BLK // 16 for c in CT_R]
    S_TOT = sum(S_R)
    OFF_S = np.concatenate([[0], np.cumsum(S_R)]).astype(int)
    OFF_C = np.concatenate([[0], np.cumsum(CT_R)]).astype(int)

    nc = bacc.Bacc("TRN2", target_bir_lowering=False, debug=False,
                   num_devices=NCORES)

    # kernel I/O
    featT = nc.dram_tensor("featT", [kdim, NPAD], F32, kind="ExternalInput").ap()
    feataT = nc.dram_tensor("feataT", [kdim, NPAD], F32, kind="ExternalInput").ap()
    idx16 = nc.dram_tensor("idx16", [NB, BLK, S_TOT], I16,
                           kind="ExternalInput").ap()
    lrow1 = nc.dram_tensor("lrow1", [NB, BLK, CT], DT1, kind="ExternalInput").ap()
    vals1 = nc.dram_tensor("vals1", [NB, BLK, CT], DT1, kind="ExternalInput").ap()
    lrow2 = nc.dram_tensor("lrow2", [NB, BLK, CT], DT2, kind="ExternalInput").ap()
    vals2 = nc.dram_tensor("vals2", [NB, BLK, CT], DT2, kind="ExternalInput").ap()
    w1 = nc.dram_tensor("w1", [kdim, F], F32, kind="ExternalInput").ap()
    wt2 = nc.dram_tensor("wt2", [BLK, F], F32, kind="ExternalInput").ap()
    bcol = nc.dram_tensor("bcol", [BLK, 1], F32, kind="ExternalInput").ap()
    iota_in = nc.dram_tensor("iota_in", [BLK, BLK], F32,
                             kind="ExternalInput").ap()
    ident_in = nc.dram_tensor("ident_in", [BLK, BLK], F32,
                              kind="ExternalInput").ap()

    z_out = nc.dram_tensor("z_out", [NPAD, F], F32, kind="ExternalOutput").ap()
    emb_out = nc.dram_tensor("emb_out", [NPAD, F], F32, kind="ExternalOutput").ap()
    ret_out = nc.dram_tensor("ret_out", [NPAD, 2], F32, kind="ExternalOutput").ap()
    reta_out = nc.dram_tensor("reta_out", [NPAD, 2], F32,
                              kind="ExternalOutput").ap()

    with tile.TileContext(nc) as tc, ExitStack() as top:
        dram = top.enter_context(tc.tile_pool(name="dram", bufs=1, space="DRAM"))
        consts = top.enter_context(tc.tile_pool(name="consts", bufs=1))

        lib = nc.gpsimd.load_library(_mlp_lib)

        def gather_rows(gat, idx_t, table):
            """Per-range gathers of one destination block into `gat`."""
            for r in range(NR):
                ni = CT_R[r] * BLK
                o0 = int(OFF_C[r]) * 2 * F
                gi = nc.gpsimd.dma_gather(
                    out_ap=gat[:, o0:o0 + CT_R[r] * 2 * F]
                    .rearrange("p (c f) -> p c f", f=2 * F),
                    in_ap=table[r * RANGE_ROWS:
                                min((r + 1) * RANGE_ROWS, NFULL), :],
                    idxs_ap=idx_t[:, OFF_S[r]:OFF_S[r] + S_R[r]],
                    num_idxs=ni, num_idxs_reg=ni, elem_size=2 * F,
                    single_packet=False)
                add_dep_helper(gi.ins, lib.ins, reason="mlp lib before gather")

        # internal DRAM: collective bounce buffers + full gather tables
        t12_in = dram.tile([NPAD, 2 * F], DT1, name="t12_in")
        t12_full = dram.tile([NFULL, 2 * F], DT1, name="t12_full")
        temb_in = dram.tile([NPAD, 2 * F], DT2, name="temb_in")
        temb_full = dram.tile([NFULL, 2 * F], DT2, name="temb_full")
        rs_dram = dram.tile([NPAD, 1], F32, name="rs_dram")
        # fp32 staging of emb|emb_a for the bilinear reload when DT2 is bf16
        embf = temb_in if DT2 == F32 else dram.tile([NPAD, 2 * F], F32,
                                                    name="embf")

        # constants
        ident = consts.tile([BLK, BLK], F32, name="ident")
        nc.sync.dma_start(out=ident[:], in_=ident_in)
        iota_1 = consts.tile([BLK, BLK], DT1, name="iota_1")
        nc.sync.dma_start(out=iota_1[:], in_=iota_in) if DT1 == F32 else None
        iota_f = None
        if DT1 != F32 or DT2 != F32:
            iota_f = consts.tile([BLK, BLK], F32, name="iota_f")
            nc.sync.dma_start(out=iota_f[:], in_=iota_in)
        if DT1 != F32:
            nc.vector.tensor_copy(iota_1[:], iota_f[:])
        if DT2 == DT1:
            iota_2 = iota_1
        else:
            iota_2 = consts.tile([BLK, BLK], DT2, name="iota_2")
            if DT2 == F32:
                nc.sync.dma_start(out=iota_2[:], in_=iota_in)
            else:
                nc.vector.tensor_copy(iota_2[:], iota_f[:])
        ones1 = consts.tile([BLK, 1], DT1, name="ones1")
        nc.vector.memset(ones1[:], 1.0)
        w1_sb = consts.tile([BLK, KC, F], F32, name="w1_sb")
        nc.sync.dma_start(out=w1_sb[:], in_=w1.rearrange("(k p) f -> p k f",
                                                         p=BLK))
        wt2_sb = consts.tile([BLK, F], F32, name="wt2_sb")
        nc.sync.dma_start(out=wt2_sb[:], in_=wt2)
        bcol_sb = consts.tile([BLK, 1], F32, name="bcol_sb")
        nc.sync.dma_start(out=bcol_sb[:], in_=bcol)

        # ---------------- phase 0: x = feat @ W1 (per shard), both tables
        SBW = 1024  # n-columns of featT loaded per DMA
        with tc.tile_pool(name="p0", bufs=3) as p0, \
             tc.tile_pool(name="p0ps", bufs=4, space="PSUM") as p0ps:
            for t_i, src in enumerate((featT, feataT)):
                for s0 in range(0, NPAD, SBW):
                    sw = min(SBW, NPAD - s0)
                    fts = []
                    for kc in range(KC):
                        ft = p0.tile([BLK, sw], F32, name=f"ft{kc}",
                                     tag=f"ft{kc}", padded_shape=[BLK, SBW])
                        nc.sync.dma_start(
                            out=ft[:],
                            in_=src[kc * BLK:(kc + 1) * BLK, s0:s0 + sw])
                        fts.append(ft)
                    for nb0 in range(0, sw, BLK):
                        ps = p0ps.tile([BLK, F], F32, name="ps", tag="ps")
                        for kc in range(KC):
                            nc.tensor.matmul(
                                ps[:], lhsT=fts[kc][:, nb0:nb0 + BLK],
                                rhs=w1_sb[:, kc, :],
                                start=(kc == 0), stop=(kc == KC - 1))
                        xt = p0.tile([BLK, F], DT1, name="xt", tag="xt")
                        nc.vector.tensor_copy(xt[:], ps[:])
                        r0 = s0 + nb0
                        nc.sync.dma_start(
                            out=t12_in[r0:r0 + BLK, t_i * F:(t_i + 1) * F],
                            in_=xt[:])

        nc.gpsimd.collective_compute(
            "AllGather", mybir.AluOpType.bypass,
            replica_groups=[list(range(NCORES))],
            ins=[t12_in.opt()], outs=[t12_full.opt()])

        # ---------------- phase 1: z|z_a = S @ (x1|x2), rowsum
        with tc.tile_pool(name="p1", bufs=3) as p1, \
             tc.tile_pool(name="p1g", bufs=2) as p1g, \
             tc.tile_pool(name="p1ps", bufs=2, space="PSUM") as p1ps:
            for b in range(NB if phases[1] else 0):
                idx_t = p1.tile([BLK, S_TOT], I16, name="idx_t", tag="idx")
                nc.sync.dma_start(out=idx_t[:], in_=idx16[b])
                lr_t = p1.tile([BLK, CT], DT1, name="lr_t", tag="lr")
                nc.sync.dma_start(out=lr_t[:], in_=lrow1[b])
                vl_t = p1.tile([BLK, CT], DT1, name="vl_t", tag="vl")
                nc.sync.dma_start(out=vl_t[:], in_=vals1[b])

                gat = p1g.tile([BLK, CT * 2 * F], DT1, name="gat", tag="gat")
                gather_rows(gat, idx_t, t12_full)

                oh = p1g.tile([BLK, CT * BLK], DT1, name="oh", tag="oh")
                oh3 = oh.rearrange("p (c r) -> p c r", r=BLK)
                nc.vector.tensor_tensor(
                    out=oh3,
                    in0=iota_1[:].unsqueeze(1).broadcast_to([BLK, CT, BLK]),
                    in1=lr_t[:].unsqueeze(2).broadcast_to([BLK, CT, BLK]),
                    op=mybir.AluOpType.is_equal)
                nc.vector.tensor_tensor(
                    out=oh3, in0=oh3,
                    in1=vl_t[:].unsqueeze(2).broadcast_to([BLK, CT, BLK]),
                    op=mybir.AluOpType.mult)

                ps = p1ps.tile([BLK, 2 * F], F32, name="psz", tag="psz")
                ps_rs = p1ps.tile([BLK, 1], F32, name="psrs", tag="psrs")
                for j in range(CT):
                    nc.tensor.matmul(
                        ps[:], lhsT=oh[:, j * BLK:(j + 1) * BLK],
                        rhs=gat[:, j * 2 * F:(j + 1) * 2 * F],
                        start=(j == 0), stop=(j == CT - 1))
                    nc.tensor.matmul(
                        ps_rs[:], lhsT=oh[:, j * BLK:(j + 1) * BLK],
                        rhs=ones1[:],
                        start=(j == 0), stop=(j == CT - 1))

                r0 = b * BLK
                zt = p1.tile([BLK, F], F32, name="zt", tag="zt")
                nc.vector.tensor_copy(zt[:], ps[:, 0:F])
                nc.sync.dma_start(out=z_out[r0:r0 + BLK, :], in_=zt[:])
                rst = p1.tile([BLK, 1], F32, name="rst", tag="rst")
                nc.vector.tensor_copy(rst[:], ps_rs[:])
                nc.sync.dma_start(out=rs_dram[r0:r0 + BLK, :], in_=rst[:])
                embt = p1.tile([BLK, 2 * F], DT2, name="embt", tag="embt")
                nc.scalar.activation(embt[:], ps[:],
                                     mybir.ActivationFunctionType.Relu)
                nc.sync.dma_start(out=temb_in[r0:r0 + BLK, :], in_=embt[:])
                if DT2 == F32:
                    nc.sync.dma_start(out=emb_out[r0:r0 + BLK, :],
                                      in_=embt[:, 0:F])
                else:
                    embtf = p1.tile([BLK, 2 * F], F32, name="embtf",
                                    tag="embtf")
                    nc.scalar.activation(embtf[:], ps[:],
                                         mybir.ActivationFunctionType.Relu)
                    nc.sync.dma_start(out=embf[r0:r0 + BLK, :], in_=embtf[:])
                    nc.sync.dma_start(out=emb_out[r0:r0 + BLK, :],
                                      in_=embtf[:, 0:F])

        nc.gpsimd.collective_compute(
            "AllGather", mybir.AluOpType.bypass,
            replica_groups=[list(range(NCORES))],
            ins=[temb_in.opt()], outs=[temb_full.opt()])

        # ---------------- phase 2: vsum | vsum_a, readout, bilinear
        with tc.tile_pool(name="p2", bufs=3) as p2, \
             tc.tile_pool(name="p2g", bufs=2) as p2g, \
             tc.tile_pool(name="p2ps", bufs=2, space="PSUM") as p2ps, \
             tc.tile_pool(name="p2bp", bufs=2, space="PSUM") as p2bp:
            for b in range(NB if phases[2] else 0):
                idx_t = p2.tile([BLK, S_TOT], I16, name="idx_t2", tag="idx")
                nc.sync.dma_start(out=idx_t[:], in_=idx16[b])
                lr_t = p2.tile([BLK, CT], DT2, name="lr_t2", tag="lr")
                nc.sync.dma_start(out=lr_t[:], in_=lrow2[b])
                vl_t = p2.tile([BLK, CT], DT2, name="vl_t2", tag="vl")
                nc.sync.dma_start(out=vl_t[:], in_=vals2[b])

                gat = p2g.tile([BLK, CT * 2 * F], DT2, name="gat2", tag="gat")
                gather_rows(gat, idx_t, temb_full)

                oh = p2g.tile([BLK, CT * BLK], DT2, name="oh2", tag="oh")
                oh3 = oh.rearrange("p (c r) -> p c r", r=BLK)
                nc.vector.tensor_tensor(
                    out=oh3,
                    in0=iota_2[:].unsqueeze(1).broadcast_to([BLK, CT, BLK]),
                    in1=lr_t[:].unsqueeze(2).broadcast_to([BLK, CT, BLK]),
                    op=mybir.AluOpType.is_equal)
                nc.vector.tensor_tensor(
                    out=oh3, in0=oh3,
                    in1=vl_t[:].unsqueeze(2).broadcast_to([BLK, CT, BLK]),
                    op=mybir.AluOpType.mult)

                ps = p2ps.tile([BLK, 2 * F], F32, name="psv", tag="psv")
                for j in range(CT):
                    nc.tensor.matmul(
                        ps[:], lhsT=oh[:, j * BLK:(j + 1) * BLK],
                        rhs=gat[:, j * 2 * F:(j + 1) * 2 * F],
                        start=(j == 0), stop=(j == CT - 1))

                r0 = b * BLK
                if p2lvl == 1:
                    jt = p2.tile([BLK, 2], F32, name="jt", tag="rett")
                    nc.vector.tensor_copy(jt[:], ps[:, 0:2])
                    nc.sync.dma_start(out=ret_out[r0:r0 + BLK, :], in_=jt[:])
                    continue
                # readout: g = sigmoid(l2norm(vsum / rowsum))
                rsl = p2.tile([BLK, 1], F32, name="rsl", tag="rsl")
                nc.sync.dma_start(out=rsl[:], in_=rs_dram[r0:r0 + BLK, :])
                rsx = p2.tile([BLK, 1], F32, name="rsx", tag="rsx")
                nc.vector.tensor_scalar_max(rsx[:], rsl[:], 1e-30)
                inv = p2.tile([BLK, 1], F32, name="inv", tag="inv")
                nc.vector.reciprocal(inv[:], rsx[:])
                gv = p2.tile([BLK, 2 * F], F32, name="gv", tag="gv")
                nc.vector.tensor_scalar_mul(gv[:], ps[:], inv[:])
                sq = p2.tile([BLK, 2 * F], F32, name="sq", tag="sq")
                nc.vector.tensor_tensor(sq[:], gv[:], gv[:],
                                        op=mybir.AluOpType.mult)
                ss = p2.tile([BLK, 2], F32, name="ss", tag="ss")
                nc.vector.tensor_reduce(
                    ss[:], sq.rearrange("p (t f) -> p t f", f=F),
                    axis=mybir.AxisListType.X, op=mybir.AluOpType.add)
                nrm = p2.tile([BLK, 2], F32, name="nrm", tag="nrm")
                nc.scalar.activation(nrm[:], ss[:],
                                     mybir.ActivationFunctionType.Sqrt)
                nc.vector.tensor_scalar_max(nrm[:], nrm[:], EPS)
                rinv = p2.tile([BLK, 2], F32, name="rinv", tag="rinv")
                nc.vector.reciprocal(rinv[:], nrm[:])
                gb = p2.tile([BLK, 2 * F], F32, name="gb", tag="gb")
                nc.scalar.activation(gb[:, 0:F], gv[:, 0:F],
                                     mybir.ActivationFunctionType.Sigmoid,
                                     scale=rinv[:, 0:1])
                nc.scalar.activation(gb[:, F:2 * F], gv[:, F:2 * F],
                                     mybir.ActivationFunctionType.Sigmoid,
                                     scale=rinv[:, 1:2])

                if p2lvl == 2:
                    jt = p2.tile([BLK, 2], F32, name="jt2", tag="rett")
                    nc.vector.tensor_copy(jt[:], gb[:, 0:2])
                    nc.sync.dma_start(out=ret_out[r0:r0 + BLK, :], in_=jt[:])
                    continue
                # bilinear: Wg = disc_w @ g[n], ret = rowsum(emb * Wg) + b
                tp1 = p2bp.tile([BLK, BLK], F32, name="tp1", tag="tp1")
                nc.tensor.transpose(tp1[:], gb[:], ident[:])
                gT = p2.tile([BLK, BLK], F32, name="gT", tag="gT")
                nc.vector.tensor_copy(gT[:], tp1[:])
                wgp = p2bp.tile([BLK, BLK], F32, name="wgp", tag="wgp")
                nc.tensor.matmul(wgp[0:F, :], lhsT=wt2_sb[0:F, :],
                                 rhs=gT[0:F, :], start=True, stop=True)
                nc.tensor.matmul(wgp[F:BLK, :], lhsT=wt2_sb[F:BLK, :],
                                 rhs=gT[F:BLK, :], start=True, stop=True)
                if p2lvl == 21:
                    jt = p2.tile([BLK, 2], F32, name="jt3", tag="rett")
                    nc.vector.tensor_copy(jt[:], gT[:, 0:2])
                    nc.sync.dma_start(out=ret_out[r0:r0 + BLK, :], in_=jt[:])
                    continue
                wgT = p2.tile([BLK, BLK], F32, name="wgT", tag="wgT")
                nc.vector.tensor_copy(wgT[:], wgp[:])
                tp2 = p2bp.tile([BLK, BLK], F32, name="tp2", tag="tp2")
                nc.tensor.transpose(tp2[:], wgT[:], ident[:])
                if p2lvl == 22:
                    jt = p2.tile([BLK, 2], F32, name="jt4", tag="rett")
                    nc.vector.tensor_copy(jt[:], wgT[:, 0:2])
                    nc.sync.dma_start(out=ret_out[r0:r0 + BLK, :], in_=jt[:])
                    continue
                wg = p2.tile([BLK, BLK], F32, name="wg", tag="wg")
                nc.vector.tensor_copy(wg[:], tp2[:])

                embt = p2.tile([BLK, 2 * F], F32, name="embt2", tag="embt")
                nc.sync.dma_start(out=embt[:], in_=embf[r0:r0 + BLK, :])

                rett = p2.tile([BLK, 2], F32, name="rett", tag="rett")
                retat = p2.tile([BLK, 2], F32, name="retat", tag="retat")
                prod = p2.tile([BLK, 2 * F], F32, name="prod", tag="prod")
                prod3 = prod.rearrange("p (t f) -> p t f", f=F)
                # red[t] = rowsum(embt_half_t * Wg_half): (emb, emb_a) order
                ein = embt.rearrange("p (t f) -> p t f", f=F)
                for dst, w_sl, swap in ((rett, slice(0, F), False),
                                        (retat, slice(F, 2 * F), True)):
                    win = wg[:, w_sl].unsqueeze(1).broadcast_to([BLK, 2, F])
                    nc.vector.tensor_tensor(out=prod3, in0=ein, in1=win,
                                            op=mybir.AluOpType.mult)
                    red = p2.tile([BLK, 2], F32, name="red", tag="red")
                    nc.vector.tensor_reduce(red[:], prod3,
                                            axis=mybir.AxisListType.X,
                                            op=mybir.AluOpType.add)
                    # ret=(emb.Wg, emb_a.Wg); ret_a=(emb_a.Wg_a, emb.Wg_a)
                    for di, ri in ((0, 1), (1, 0)) if swap else ((0, 0),
                                                                (1, 1)):
                        nc.vector.tensor_scalar(
                            out=dst[:, di:di + 1], in0=red[:, ri:ri + 1],
                            scalar1=bcol_sb[:], scalar2=None,
                            op0=mybir.AluOpType.add)
                nc.sync.dma_start(out=ret_out[r0:r0 + BLK, :], in_=rett[:])
                nc.sync.dma_start(out=reta_out[r0:r0 + BLK, :], in_=retat[:])

    nc.compile()
    return nc


# ------------------------------------------------------------------- running

def _make_runner(nc, n_cores):
    """Replicates bass2jax.run_bass_via_pjrt's jit construction, returning a
    callable we can invoke repeatedly (for timing) with pre-staged inputs."""
    import jax
    from jax.sharding import Mesh, PartitionSpec, NamedSharding
    from jax.experimental.shard_map import shard_map
    from concourse import bass2jax

    bass2jax.install_neuronx_cc_hook()

    partition_name = (nc.partition_id_tensor.name
                      if nc.partition_id_tensor else None)
    in_names, out_names, out_avals, zero_outs = [], [], [], []
    for alloc in nc.m.functions[0].allocations:
        if not isinstance(alloc, mybir.MemoryLocationSet):
            continue
        name = alloc.memorylocations[0].name
        if alloc.kind == "ExternalInput":
            if name != partition_name:
                in_names.append(name)
        elif alloc.kind == "ExternalOutput":
            shape = tuple(alloc.tensor_shape)
            dtype = mybir.dt.np(alloc.dtype)
            out_names.append(name)
            out_avals.append(jax.core.ShapedArray(shape, dtype))
            zero_outs.append(np.zeros(shape, dtype))
    n_params = len(in_names)
    n_outs = len(out_avals)
    in_names = in_names + out_names
    if partition_name is not None:
        in_names.append(partition_name)
    donate = tuple(range(n_params, n_params + n_outs))

    def _body(*args):
        operands = list(args)
        if partition_name is not None:
            operands.append(bass2jax.partition_id_tensor())
        outs = bass2jax._bass_exec_p.bind(
            *operands,
            out_avals=tuple(out_avals),
            in_names=tuple(in_names),
            out_names=tuple(out_names),
            lowering_input_output_aliases=(),
            sim_require_finite=True,
            sim_require_nnan=True,
            nc=nc,
        )
        return tuple(outs)

    devices = jax.devices()[:n_cores]
    mesh = Mesh(np.asarray(devices), ("core",))
    sharded = jax.jit(
        shard_map(_body, mesh=mesh,
                  in_specs=(PartitionSpec("core"),) * (n_params + n_outs),
                  out_specs=(PartitionSpec("core"),) * n_outs,
                  check_rep=False),
        donate_argnums=donate, keep_unused=True)
    shard_spec = NamedSharding(mesh, PartitionSpec("core"))
    return sharded, in_names[:n_params], out_names, out_avals, zero_outs, \
        shard_spec


def _run(nc, in_maps, n_cores, time_iters=0):
    import jax
    import time as _time

    sharded, in_names, out_names, out_avals, zero_outs, shard_spec = \
        _make_runner(nc, n_cores)
    concat_in = [
        np.concatenate([np.asarray(in_maps[c][n]) for c in range(n_cores)],
                       axis=0)
        for n in in_names
    ]
    staged_in = [jax.device_put(a, shard_spec) for a in concat_in]
    jax.block_until_ready(staged_in)

    def make_zeros():
        zs = [jax.device_put(
            np.zeros((n_cores * z.shape[0], *z.shape[1:]), z.dtype),
            shard_spec) for z in zero_outs]
        jax.block_until_ready(zs)
        return zs

    out_arrs = sharded(*staged_in, *make_zeros())
    jax.block_until_ready(out_arrs)

    times = []
    for _ in range(time_iters):
        zs = make_zeros()
        t0 = _time.perf_counter()
        out_arrs2 = sharded(*staged_in, *zs)
        jax.block_until_ready(out_arrs2)
        times.append(_time.perf_counter() - t0)

    results = [
        {name: np.asarray(out_arrs[i]).reshape(n_cores, *out_avals[i].shape)[c]
         for i, name in enumerate(out_names)}
        for c in range(n_cores)
    ]
    return results, (min(times) if times else None)


def postprocess(cfg, results):
    Nc = cfg.Nc

    def gather(name):
        return np.concatenate([results[c][name][:Nc] for c in range(NCORES)],
                              axis=0)

    return (gather("z_out"), gather("emb_out"), gather("ret_out"),
            gather("reta_out"))


def kernel(**inputs):
    cfg, in_maps = preprocess(inputs)
    nc = build_program(cfg)
    results, _ = _run(nc, in_maps, NCORES, time_iters=0)
    return postprocess(cfg, results)


def kernel_traced(time_iters=5, tab1_bf16=TAB1_BF16, tab2_bf16=TAB2_BF16,
                  **inputs):
    """Like kernel() but also times repeated executions (wall clock)."""
    cfg, in_maps = preprocess(inputs, tab1_bf16, tab2_bf16)
    nc = build_program(cfg)
    results, best = _run(nc, in_maps, NCORES, time_iters=time_iters)
    return postprocess(cfg, results), best


# revision 17
# speedup vs baseline: 1.1606x; 1.1606x over previous
"""Trainium2 Bass kernel for nn_Encoder_3796751090357 (GNN message passing).

Reference computation (see reference.py):
    x1   = feat   @ W1                      [N, 64]
    z    = S @ x1        (S = sparse adj)   [N, 64]   -> output "hidden_emb"
    emb  = relu(z)                                    -> output "emb"
    x2   = feat_a @ W1
    z_a  = S @ x2 ; emb_a = relu(z_a)
    g    = sigmoid(l2norm((S @ emb)   / rowsum(S)))
    g_a  = sigmoid(l2norm((S @ emb_a) / rowsum(S)))
    ret   = [bilin(emb, g),   bilin(emb_a, g)]        -> output [N, 2]
    ret_a = [bilin(emb_a, g_a), bilin(emb, g_a)]      -> output [N, 2]
    (the reference's `h` tensor is computed but unused -> skipped here)

Sharding: nodes (destination rows) are sharded across the 8 cores; edges are
partitioned by destination row.  Each core computes x1/x2 for its node shard;
an AllGather materializes the full [100352, 128] gather table in every core's
DRAM; each core then runs the SpMMs for its destination shard as one-hot
matmuls: for each 128-row destination block, edges are packed into chunks of
128; each chunk contributes onehot[e, r] = val_e * (lrow_e == r) as PE
weights against the dma_gather'ed source rows as the moving operand,
accumulating the block in PSUM.

The per-edge source rows are fetched with the GPSIMD dma_gather custom op
(mlp ucode library; int16 indices), so the table is split into 4 ranges of
2^15 rows; each destination block issues one gather per range.  Edge slots
are ordered (range, chunk, partition) to match dma_gather's output layout
(index k -> partition k%128, column k//128).
"""

from contextlib import ExitStack

import numpy as np
import ml_dtypes

import concourse.bacc as bacc
import concourse.bass as bass
import concourse.mybir as mybir
import concourse.tile as tile
from concourse.library_config import mlp as _mlp_lib
from concourse.tile import add_dep_helper
from concourse.masks import make_identity  # noqa: F401  (kept for reference)

F32 = mybir.dt.float32
BF16 = mybir.dt.bfloat16
I16 = mybir.dt.int16

NCORES = 8
BLK = 128
KDIM = 512  # IN_F
F = 64      # OUT_F
EPS = 1e-12
RANGE_BITS = 15  # int16 gather-index limit (2**15 rows per range)
RANGE_ROWS = 1 << RANGE_BITS

# gather-table dtypes: phase 1 (x1|x2 -> z) and phase 2 (emb|emb_a -> vsum)
TAB1_BF16 = False
TAB2_BF16 = False


class Cfg:
    def __init__(self, n_nodes, ct_r, tab1_bf16, tab2_bf16):
        self.RNG_ROWS = 0  # set by preprocess/build
        self.N = n_nodes
        assert n_nodes % NCORES == 0
        self.Nc = n_nodes // NCORES            # real rows per core
        self.NB = -(-self.Nc // BLK)           # dest blocks per core
        self.NPAD = self.NB * BLK              # padded rows per core
        self.NFULL = self.NPAD * NCORES        # padded rows, all cores
        self.CT_R = list(ct_r)                 # chunks per block, per range
        self.CT = sum(ct_r)                    # total chunks per block
        self.NR = len(ct_r)
        self.tab1_bf16 = tab1_bf16
        self.tab2_bf16 = tab2_bf16


def _npdt(bf16):
    return ml_dtypes.bfloat16 if bf16 else np.float32


# ----------------------------------------------------------------- host side

def preprocess(inputs, tab1_bf16=TAB1_BF16, tab2_bf16=TAB2_BF16):
    """Sort/pad edges, pre-transpose features, build per-core input maps."""
    feat = np.ascontiguousarray(np.asarray(inputs["feat"], dtype=np.float32))
    feat_a = np.ascontiguousarray(np.asarray(inputs["feat_a"], dtype=np.float32))
    vals = np.asarray(inputs["adj_vals"], dtype=np.float32)
    rows = np.asarray(inputs["adj_rows"]).astype(np.int64)
    cols = np.asarray(inputs["adj_cols"]).astype(np.int64)
    w1 = np.ascontiguousarray(np.asarray(inputs["weight1"], dtype=np.float32))
    disc_w = np.asarray(inputs["disc_w"], dtype=np.float32)
    disc_b = np.float32(np.asarray(inputs["disc_b"]))

    n_nodes = feat.shape[0]
    assert n_nodes % NCORES == 0
    nc_rows = n_nodes // NCORES
    nb = -(-nc_rows // BLK)
    npad = nb * BLK
    nfull = npad * NCORES
    n_ranges = -(-nfull // RANGE_ROWS)
    rng_rows = -(-nfull // n_ranges)  # equal-size ranges (<= RANGE_ROWS)

    # padded source-row id in the allgathered table + its range
    pid = (cols // nc_rows) * npad + cols % nc_rows
    rng_id = pid // rng_rows

    shard = rows // nc_rows
    lrow_core = rows - shard * nc_rows
    pblock = shard * nb + lrow_core // BLK        # global dest block id
    lr_all = (lrow_core % BLK).astype(np.float32)

    # order edges by (dest block, source range)
    key = pblock * n_ranges + rng_id
    order = np.argsort(key, kind="stable")
    key_s = key[order]
    pid_s = pid[order]
    vals_s = vals[order]
    lr_s = lr_all[order]

    n_groups = NCORES * nb * n_ranges
    counts = np.bincount(key_s, minlength=n_groups)
    cnt_br = counts.reshape(NCORES * nb, n_ranges)
    ct_r = [int(-(-cnt_br[:, r].max() // BLK)) for r in range(n_ranges)]
    ct_r = [max(c, 1) for c in ct_r]
    cfg = Cfg(n_nodes, ct_r, tab1_bf16, tab2_bf16)
    cfg.RNG_ROWS = rng_rows
    ct_tot = cfg.CT
    off_c = np.concatenate([[0], np.cumsum(ct_r)])  # chunk offset per range

    starts = np.zeros(n_groups, dtype=np.int64)
    np.cumsum(counts[:-1], out=starts[1:])
    rank = np.arange(len(key_s), dtype=np.int64) - starts[key_s]

    g_rng = key_s % n_ranges
    g_blk = key_s // n_ranges          # global block id (core*nb + b)
    ci = g_blk // nb
    bi = g_blk % nb

    # int16 gather indices: wrapped [16, S] layout, k -> (k%16, k//16);
    # per-range blocks concatenated along S. (S units of 16 idxs.)
    s_r = [c * BLK // 16 for c in ct_r]
    off_s = np.concatenate([[0], np.cumsum(s_r)])
    s_tot = int(off_s[-1])
    idx16 = np.full((NCORES, nb, 16, s_tot), -1, dtype=np.int16)
    idx16[ci, bi, rank % 16, off_s[g_rng] + rank // 16] = \
        (pid_s - g_rng * rng_rows).astype(np.int16)
    idx16 = np.ascontiguousarray(np.tile(idx16, (1, 1, 8, 1)))
    cnts = np.ascontiguousarray(
        cnt_br.reshape(NCORES, nb * n_ranges)[:, None, :].astype(np.int32))

    # one-hot operands: slot (p, chunk) with chunk = off_c[range] + rank//128
    lrowf = np.zeros((NCORES, nb, BLK, ct_tot), dtype=np.float32)
    valsf = np.zeros((NCORES, nb, BLK, ct_tot), dtype=np.float32)
    cslot = off_c[g_rng] + rank // BLK
    lrowf[ci, bi, rank % BLK, cslot] = lr_s
    valsf[ci, bi, rank % BLK, cslot] = vals_s

    # pre-transposed feature shards [KDIM, NPAD]
    fT = np.ascontiguousarray(feat.T)
    faT = np.ascontiguousarray(feat_a.T)
    k = feat.shape[1]

    wt2 = np.ascontiguousarray(
        np.concatenate([disc_w.T, disc_w.T], axis=0).astype(np.float32))
    bcol = np.full((BLK, 1), disc_b, dtype=np.float32)
    iota = np.broadcast_to(np.arange(BLK, dtype=np.float32)[None, :],
                           (BLK, BLK)).copy()
    ident = np.eye(BLK, dtype=np.float32)

    d1 = _npdt(tab1_bf16)
    d2 = _npdt(tab2_bf16)
    lv = np.ascontiguousarray(
        np.concatenate([lrowf, valsf], axis=3))  # [NC, nb, BLK, 2*CT]

    in_maps = []
    for c in range(NCORES):
        sl = slice(c * nc_rows, (c + 1) * nc_rows)
        ft = np.zeros((k, npad), dtype=np.float32)
        ft[:, :nc_rows] = fT[:, sl]
        fat = np.zeros((k, npad), dtype=np.float32)
        fat[:, :nc_rows] = faT[:, sl]
        in_maps.append({
            "featT": ft,
            "feataT": fat,
            "idx16": idx16[c],
            "lv": lv[c],
            "cnts": cnts[c],
            "w1": w1,
            "wt2": wt2,
            "bcol": bcol,
            "iota_in": iota,
            "ident_in": ident,
        })
    return cfg, in_maps


# --------------------------------------------------------------- device side

def build_program(cfg, kdim=KDIM, phases=(1, 1, 1), p2lvl=3):
    NB, CT, NPAD, NFULL = cfg.NB, cfg.CT, cfg.NPAD, cfg.NFULL
    CT_R, NR = cfg.CT_R, cfg.NR
    KC = kdim // BLK  # contraction chunks for the dense matmul
    DT1 = BF16 if cfg.tab1_bf16 else F32
    DT2 = BF16 if cfg.tab2_bf16 else F32
    RR = cfg.RNG_ROWS or RANGE_ROWS
    S_R = [c * BLK // 16 for c in CT_R]
    S_TOT = sum(S_R)
    OFF_S = np.concatenate([[0], np.cumsum(S_R)]).astype(int)
    OFF_C = np.concatenate([[0], np.cumsum(CT_R)]).astype(int)

    nc = bacc.Bacc("TRN2", target_bir_lowering=False, debug=False,
                   num_devices=NCORES)

    # kernel I/O
    featT = nc.dram_tensor("featT", [kdim, NPAD], F32, kind="ExternalInput").ap()
    feataT = nc.dram_tensor("feataT", [kdim, NPAD], F32, kind="ExternalInput").ap()
    idx16 = nc.dram_tensor("idx16", [NB, BLK, S_TOT], I16,
                           kind="ExternalInput").ap()
    lv = nc.dram_tensor("lv", [NB, BLK, 2 * CT], F32,
                        kind="ExternalInput").ap()
    cnts = nc.dram_tensor("cnts", [1, NB * NR], mybir.dt.int32,
                          kind="ExternalInput").ap()
    w1 = nc.dram_tensor("w1", [kdim, F], F32, kind="ExternalInput").ap()
    wt2 = nc.dram_tensor("wt2", [BLK, F], F32, kind="ExternalInput").ap()
    bcol = nc.dram_tensor("bcol", [BLK, 1], F32, kind="ExternalInput").ap()
    iota_in = nc.dram_tensor("iota_in", [BLK, BLK], F32,
                             kind="ExternalInput").ap()
    ident_in = nc.dram_tensor("ident_in", [BLK, BLK], F32,
                              kind="ExternalInput").ap()

    z_out = nc.dram_tensor("z_out", [NPAD, F], F32, kind="ExternalOutput").ap()
    emb_out = nc.dram_tensor("emb_out", [NPAD, F], F32, kind="ExternalOutput").ap()
    ret_out = nc.dram_tensor("ret_out", [NPAD, 2], F32, kind="ExternalOutput").ap()
    reta_out = nc.dram_tensor("reta_out", [NPAD, 2], F32,
                              kind="ExternalOutput").ap()

    with tile.TileContext(nc) as tc, ExitStack() as top:
        dram = top.enter_context(tc.tile_pool(name="dram", bufs=1, space="DRAM"))
        consts = top.enter_context(tc.tile_pool(name="consts", bufs=1))

        lib = nc.gpsimd.load_library(_mlp_lib)
        reg_ni = top.enter_context(nc.gpsimd.register("reg_ni"))

        def gather_rows(gat, idx_t, table, cnts_sb, b):
            """Per-range gathers of one destination block into `gat`."""
            for r in range(NR):
                ni = CT_R[r] * BLK
                o0 = int(OFF_C[r]) * 2 * F
                nc.gpsimd.reg_load(reg_ni,
                                   cnts_sb[0:1, b * NR + r:b * NR + r + 1])
                gi = nc.gpsimd.dma_gather(
                    out_ap=gat[:, o0:o0 + CT_R[r] * 2 * F]
                    .rearrange("p (c f) -> p c f", f=2 * F),
                    in_ap=table[r * RR:min((r + 1) * RR, NFULL), :],
                    idxs_ap=idx_t[:, OFF_S[r]:OFF_S[r] + S_R[r]],
                    num_idxs=ni, num_idxs_reg=reg_ni, elem_size=2 * F,
                    single_packet=False)
                add_dep_helper(gi.ins, lib.ins, reason="mlp lib before gather")

        # internal DRAM: collective bounce buffers + full gather tables
        t12_in = dram.tile([NPAD, 2 * F], DT1, name="t12_in")
        t12_full = dram.tile([NFULL, 2 * F], DT1, name="t12_full")
        temb_in = dram.tile([NPAD, 2 * F], DT2, name="temb_in")
        temb_full = dram.tile([NFULL, 2 * F], DT2, name="temb_full")
        rs_dram = dram.tile([NPAD, 1], F32, name="rs_dram")
        # fp32 staging of emb|emb_a for the bilinear reload when DT2 is bf16
        embf = temb_in if DT2 == F32 else dram.tile([NPAD, 2 * F], F32,
                                                    name="embf")

        # constants
        ident = consts.tile([BLK, BLK], F32, name="ident")
        nc.sync.dma_start(out=ident[:], in_=ident_in)
        iota_1 = consts.tile([BLK, BLK], DT1, name="iota_1")
        nc.sync.dma_start(out=iota_1[:], in_=iota_in) if DT1 == F32 else None
        iota_f = None
        if DT1 != F32 or DT2 != F32:
            iota_f = consts.tile([BLK, BLK], F32, name="iota_f")
            nc.sync.dma_start(out=iota_f[:], in_=iota_in)
        if DT1 != F32:
            nc.vector.tensor_copy(iota_1[:], iota_f[:])
        if DT2 == DT1:
            iota_2 = iota_1
        else:
            iota_2 = consts.tile([BLK, BLK], DT2, name="iota_2")
            if DT2 == F32:
                nc.sync.dma_start(out=iota_2[:], in_=iota_in)
            else:
                nc.vector.tensor_copy(iota_2[:], iota_f[:])
        ones1 = consts.tile([BLK, 1], DT1, name="ones1")
        nc.vector.memset(ones1[:], 1.0)
        w1_sb = consts.tile([BLK, KC, F], F32, name="w1_sb")
        nc.sync.dma_start(out=w1_sb[:], in_=w1.rearrange("(k p) f -> p k f",
                                                         p=BLK))
        wt2_sb = consts.tile([BLK, F], F32, name="wt2_sb")
        nc.sync.dma_start(out=wt2_sb[:], in_=wt2)
        bcol_sb = consts.tile([BLK, 1], F32, name="bcol_sb")
        nc.sync.dma_start(out=bcol_sb[:], in_=bcol)
        cnts_sb = consts.tile([1, NB * NR], mybir.dt.int32, name="cnts_sb")
        nc.sync.dma_start(out=cnts_sb[:], in_=cnts)

        # ---------------- phase 0: x = feat @ W1 (per shard), both tables
        SBW = 1024  # n-columns of featT loaded per DMA
        with tc.tile_pool(name="p0", bufs=3) as p0, \
             tc.tile_pool(name="p0ps", bufs=4, space="PSUM") as p0ps:
            for t_i, src in enumerate((featT, feataT)):
                for s0 in range(0, NPAD, SBW):
                    sw = min(SBW, NPAD - s0)
                    fts = []
                    for kc in range(KC):
                        ft = p0.tile([BLK, sw], F32, name=f"ft{kc}",
                                     tag=f"ft{kc}", padded_shape=[BLK, SBW])
                        nc.sync.dma_start(
                            out=ft[:],
                            in_=src[kc * BLK:(kc + 1) * BLK, s0:s0 + sw])
                        fts.append(ft)
                    for nb0 in range(0, sw, BLK):
                        ps = p0ps.tile([BLK, F], F32, name="ps", tag="ps")
                        for kc in range(KC):
                            nc.tensor.matmul(
                                ps[:], lhsT=fts[kc][:, nb0:nb0 + BLK],
                                rhs=w1_sb[:, kc, :],
                                start=(kc == 0), stop=(kc == KC - 1))
                        xt = p0.tile([BLK, F], DT1, name="xt", tag="xt")
                        nc.vector.tensor_copy(xt[:], ps[:])
                        r0 = s0 + nb0
                        nc.sync.dma_start(
                            out=t12_in[r0:r0 + BLK, t_i * F:(t_i + 1) * F],
                            in_=xt[:])

        nc.gpsimd.collective_compute(
            "AllGather", mybir.AluOpType.bypass,
            replica_groups=[list(range(NCORES))],
            ins=[t12_in.opt()], outs=[t12_full.opt()])

        # ---------------- phase 1: z|z_a = S @ (x1|x2), rowsum
        with tc.tile_pool(name="p1", bufs=3) as p1, \
             tc.tile_pool(name="p1g", bufs=2) as p1g, \
             tc.tile_pool(name="p1ps", bufs=3, space="PSUM") as p1ps:
            NG = 3
            g1ts = []
            for i in range(NB and NG):
                g1 = p1g.tile([BLK, CT * 2 * F], DT1, name=f"g1_{i}",
                              tag=f"g1_{i}", bufs=1)
                nc.vector.memset(g1[:], 0.0)
                g1ts.append(g1)
            for b in range(NB if phases[1] else 0):
                idx_t = p1.tile([BLK, S_TOT], I16, name="idx_t", tag="idx")
                nc.sync.dma_start(out=idx_t[:], in_=idx16[b])
                lv_t = p1.tile([BLK, 2 * CT], F32, name="lv_t", tag="lv")
                nc.sync.dma_start(out=lv_t[:], in_=lv[b])

                gat = g1ts[b % NG]
                gather_rows(gat, idx_t, t12_full, cnts_sb, b)

                oh = p1g.tile([BLK, CT * BLK], DT1, name="oh", tag="oh")
                for j in range(CT):
                    nc.vector.tensor_scalar(
                        out=oh[:, j * BLK:(j + 1) * BLK], in0=iota_1[:],
                        scalar1=lv_t[:, j:j + 1],
                        scalar2=lv_t[:, CT + j:CT + j + 1],
                        op0=mybir.AluOpType.is_equal,
                        op1=mybir.AluOpType.mult)

                ps = p1ps.tile([BLK, 2 * F], F32, name="psz", tag="psz")
                ps_rs = p1ps.tile([BLK, 1], F32, name="psrs", tag="psrs")
                for j in range(CT):
                    nc.tensor.matmul(
                        ps[:], lhsT=oh[:, j * BLK:(j + 1) * BLK],
                        rhs=gat[:, j * 2 * F:(j + 1) * 2 * F],
                        start=(j == 0), stop=(j == CT - 1))
                    nc.tensor.matmul(
                        ps_rs[:], lhsT=oh[:, j * BLK:(j + 1) * BLK],
                        rhs=ones1[:],
                        start=(j == 0), stop=(j == CT - 1))

                r0 = b * BLK
                zt = p1.tile([BLK, F], F32, name="zt", tag="zt")
                nc.vector.tensor_copy(zt[:], ps[:, 0:F])
                nc.scalar.dma_start(out=z_out[r0:r0 + BLK, :], in_=zt[:])
                rst = p1.tile([BLK, 1], F32, name="rst", tag="rst")
                nc.vector.tensor_copy(rst[:], ps_rs[:])
                nc.scalar.dma_start(out=rs_dram[r0:r0 + BLK, :], in_=rst[:])
                embt = p1.tile([BLK, 2 * F], DT2, name="embt", tag="embt")
                nc.scalar.activation(embt[:], ps[:],
                                     mybir.ActivationFunctionType.Relu)
                nc.scalar.dma_start(out=temb_in[r0:r0 + BLK, :], in_=embt[:])
                if DT2 == F32:
                    nc.scalar.dma_start(out=emb_out[r0:r0 + BLK, :],
                                        in_=embt[:, 0:F])
                else:
                    embtf = p1.tile([BLK, 2 * F], F32, name="embtf",
                                    tag="embtf")
                    nc.scalar.activation(embtf[:], ps[:],
                                         mybir.ActivationFunctionType.Relu)
                    nc.scalar.dma_start(out=embf[r0:r0 + BLK, :], in_=embtf[:])
                    nc.scalar.dma_start(out=emb_out[r0:r0 + BLK, :],
                                        in_=embtf[:, 0:F])

        nc.gpsimd.collective_compute(
            "AllGather", mybir.AluOpType.bypass,
            replica_groups=[list(range(NCORES))],
            ins=[temb_in.opt()], outs=[temb_full.opt()])

        # ---------------- phase 2: vsum | vsum_a, readout, bilinear
        with tc.tile_pool(name="p2", bufs=3) as p2, \
             tc.tile_pool(name="p2g", bufs=2) as p2g, \
             tc.tile_pool(name="p2ps", bufs=4, space="PSUM") as p2ps, \
             tc.tile_pool(name="p2bp", bufs=1, space="PSUM") as p2bp:
            NG = 3
            g2ts = []
            for i in range(NB and NG):
                g2 = p2g.tile([BLK, CT * 2 * F], DT2, name=f"g2_{i}",
                              tag=f"g2_{i}", bufs=1)
                nc.vector.memset(g2[:], 0.0)
                g2ts.append(g2)
            for b in range(NB if phases[2] else 0):
                idx_t = p2.tile([BLK, S_TOT], I16, name="idx_t2", tag="idx")
                nc.sync.dma_start(out=idx_t[:], in_=idx16[b])
                lv_t = p2.tile([BLK, 2 * CT], F32, name="lv_t2", tag="lv")
                nc.sync.dma_start(out=lv_t[:], in_=lv[b])

                gat = g2ts[b % NG]
                gather_rows(gat, idx_t, temb_full, cnts_sb, b)

                oh = p2g.tile([BLK, CT * BLK], DT2, name="oh2", tag="oh")
                for j in range(CT):
                    nc.vector.tensor_scalar(
                        out=oh[:, j * BLK:(j + 1) * BLK], in0=iota_2[:],
                        scalar1=lv_t[:, j:j + 1],
                        scalar2=lv_t[:, CT + j:CT + j + 1],
                        op0=mybir.AluOpType.is_equal,
                        op1=mybir.AluOpType.mult)

                ps = p2ps.tile([BLK, 2 * F], F32, name="psv", tag="psv")
                for j in range(CT):
                    nc.tensor.matmul(
                        ps[:], lhsT=oh[:, j * BLK:(j + 1) * BLK],
                        rhs=gat[:, j * 2 * F:(j + 1) * 2 * F],
                        start=(j == 0), stop=(j == CT - 1))

                r0 = b * BLK
                if p2lvl == 1:
                    jt = p2.tile([BLK, 2], F32, name="jt", tag="rett")
                    nc.vector.tensor_copy(jt[:], ps[:, 0:2])
                    nc.sync.dma_start(out=ret_out[r0:r0 + BLK, :], in_=jt[:])
                    continue
                # readout: g = sigmoid(l2norm(vsum / rowsum))
                rsl = p2.tile([BLK, 1], F32, name="rsl", tag="rsl")
                nc.sync.dma_start(out=rsl[:], in_=rs_dram[r0:r0 + BLK, :])
                rsx = p2.tile([BLK, 1], F32, name="rsx", tag="rsx")
                nc.vector.tensor_scalar_max(rsx[:], rsl[:], 1e-30)
                inv = p2.tile([BLK, 1], F32, name="inv", tag="inv")
                nc.vector.reciprocal(inv[:], rsx[:])
                gv = p2.tile([BLK, 2 * F], F32, name="gv", tag="gv")
                nc.vector.tensor_scalar_mul(gv[:], ps[:], inv[:])
                sq = p2.tile([BLK, 2 * F], F32, name="sq", tag="sq")
                nc.vector.tensor_tensor(sq[:], gv[:], gv[:],
                                        op=mybir.AluOpType.mult)
                ss = p2.tile([BLK, 2], F32, name="ss", tag="ss")
                nc.vector.tensor_reduce(
                    ss[:], sq.rearrange("p (t f) -> p t f", f=F),
                    axis=mybir.AxisListType.X, op=mybir.AluOpType.add)
                nrm = p2.tile([BLK, 2], F32, name="nrm", tag="nrm")
                nc.scalar.activation(nrm[:], ss[:],
                                     mybir.ActivationFunctionType.Sqrt)
                nc.vector.tensor_scalar_max(nrm[:], nrm[:], EPS)
                rinv = p2.tile([BLK, 2], F32, name="rinv", tag="rinv")
                nc.vector.reciprocal(rinv[:], nrm[:])
                gb = p2.tile([BLK, 2 * F], F32, name="gb", tag="gb")
                nc.scalar.activation(gb[:, 0:F], gv[:, 0:F],
                                     mybir.ActivationFunctionType.Sigmoid,
                                     scale=rinv[:, 0:1])
                nc.scalar.activation(gb[:, F:2 * F], gv[:, F:2 * F],
                                     mybir.ActivationFunctionType.Sigmoid,
                                     scale=rinv[:, 1:2])

                if p2lvl == 2:
                    jt = p2.tile([BLK, 2], F32, name="jt2", tag="rett")
                    nc.vector.tensor_copy(jt[:], gb[:, 0:2])
                    nc.sync.dma_start(out=ret_out[r0:r0 + BLK, :], in_=jt[:])
                    continue
                # bilinear: Wg = disc_w @ g[n], ret = rowsum(emb * Wg) + b
                tp1 = p2bp.tile([BLK, BLK], F32, name="tp1", tag="tp1")
                nc.tensor.transpose(tp1[:], gb[:], ident[:])
                gT = p2.tile([BLK, BLK], F32, name="gT", tag="gT")
                nc.vector.tensor_copy(gT[:], tp1[:])
                wgp = p2bp.tile([BLK, BLK], F32, name="wgp", tag="wgp")
                nc.tensor.matmul(wgp[0:F, :], lhsT=wt2_sb[0:F, :],
                                 rhs=gT[0:F, :], start=True, stop=True)
                nc.tensor.matmul(wgp[F:BLK, :], lhsT=wt2_sb[F:BLK, :],
                                 rhs=gT[F:BLK, :], start=True, stop=True)
                if p2lvl == 21:
                    jt = p2.tile([BLK, 2], F32, name="jt3", tag="rett")
                    nc.vector.tensor_copy(jt[:], gT[:, 0:2])
                    nc.sync.dma_start(out=ret_out[r0:r0 + BLK, :], in_=jt[:])
                    continue
                wgT = p2.tile([BLK, BLK], F32, name="wgT", tag="wgT")
                nc.vector.tensor_copy(wgT[:], wgp[:])
                tp2 = p2bp.tile([BLK, BLK], F32, name="tp2", tag="tp2")
                nc.tensor.transpose(tp2[:], wgT[:], ident[:])
                if p2lvl == 22:
                    jt = p2.tile([BLK, 2], F32, name="jt4", tag="rett")
                    nc.vector.tensor_copy(jt[:], wgT[:, 0:2])
                    nc.sync.dma_start(out=ret_out[r0:r0 + BLK, :], in_=jt[:])
                    continue
                wg = p2.tile([BLK, BLK], F32, name="wg", tag="wg")
                nc.vector.tensor_copy(wg[:], tp2[:])

                embt = p2.tile([BLK, 2 * F], F32, name="embt2", tag="embt")
                nc.sync.dma_start(out=embt[:], in_=embf[r0:r0 + BLK, :])

                rett = p2.tile([BLK, 2], F32, name="rett", tag="rett")
                retat = p2.tile([BLK, 2], F32, name="retat", tag="retat")
                prod = p2.tile([BLK, 2 * F], F32, name="prod", tag="prod")
                prod3 = prod.rearrange("p (t f) -> p t f", f=F)
                # red[t] = rowsum(embt_half_t * Wg_half): (emb, emb_a) order
                ein = embt.rearrange("p (t f) -> p t f", f=F)
                for dst, w_sl, swap in ((rett, slice(0, F), False),
                                        (retat, slice(F, 2 * F), True)):
                    win = wg[:, w_sl].unsqueeze(1).broadcast_to([BLK, 2, F])
                    nc.vector.tensor_tensor(out=prod3, in0=ein, in1=win,
                                            op=mybir.AluOpType.mult)
                    red = p2.tile([BLK, 2], F32, name="red", tag="red")
                    nc.vector.tensor_reduce(red[:], prod3,
                                            axis=mybir.AxisListType.X,
                                            op=mybir.AluOpType.add)
                    # ret=(emb.Wg, emb_a.Wg); ret_a=(emb_a.Wg_a, emb.Wg_a)
                    for di, ri in ((0, 1), (1, 0)) if swap else ((0, 0),
                                                                (1, 1)):
                        nc.vector.tensor_scalar(
                            out=dst[:, di:di + 1], in0=red[:, ri:ri + 1],
                            scalar1=bcol_sb[:], scalar2=None,
                            op0=mybir.AluOpType.add)
                nc.scalar.dma_start(out=ret_out[r0:r0 + BLK, :], in_=rett[:])
                nc.scalar.dma_start(out=reta_out[r0:r0 + BLK, :], in_=retat[:])

    nc.compile()
    return nc


# ------------------------------------------------------------------- running

def _make_runner(nc, n_cores):
    """Replicates bass2jax.run_bass_via_pjrt's jit construction, returning a
    callable we can invoke repeatedly (for timing) with pre-staged inputs."""
    import jax
    from jax.sharding import Mesh, PartitionSpec, NamedSharding
    from jax.experimental.shard_map import shard_map
    from concourse import bass2jax

    bass2jax.install_neuronx_cc_hook()

    partition_name = (nc.partition_id_tensor.name
                      if nc.partition_id_tensor else None)
    in_names, out_names, out_avals, zero_outs = [], [], [], []
    for alloc in nc.m.functions[0].allocations:
        if not isinstance(alloc, mybir.MemoryLocationSet):
            continue
        name = alloc.memorylocations[0].name
        if alloc.kind == "ExternalInput":
            if name != partition_name:
                in_names.append(name)
        elif alloc.kind == "ExternalOutput":
            shape = tuple(alloc.tensor_shape)
            dtype = mybir.dt.np(alloc.dtype)
            out_names.append(name)
            out_avals.append(jax.core.ShapedArray(shape, dtype))
            zero_outs.append(np.zeros(shape, dtype))
    n_params = len(in_names)
    n_outs = len(out_avals)
    in_names = in_names + out_names
    if partition_name is not None:
        in_names.append(partition_name)
    donate = tuple(range(n_params, n_params + n_outs))

    def _body(*args):
        operands = list(args)
        if partition_name is not None:
            operands.append(bass2jax.partition_id_tensor())
        outs = bass2jax._bass_exec_p.bind(
            *operands,
            out_avals=tuple(out_avals),
            in_names=tuple(in_names),
            out_names=tuple(out_names),
            lowering_input_output_aliases=(),
            sim_require_finite=True,
            sim_require_nnan=True,
            nc=nc,
        )
        return tuple(outs)

    devices = jax.devices()[:n_cores]
    mesh = Mesh(np.asarray(devices), ("core",))
    sharded = jax.jit(
        shard_map(_body, mesh=mesh,
                  in_specs=(PartitionSpec("core"),) * (n_params + n_outs),
                  out_specs=(PartitionSpec("core"),) * n_outs,
                  check_rep=False),
        donate_argnums=donate, keep_unused=True)
    shard_spec = NamedSharding(mesh, PartitionSpec("core"))
    return sharded, in_names[:n_params], out_names, out_avals, zero_outs, \
        shard_spec


def _run(nc, in_maps, n_cores, time_iters=0):
    import jax
    import time as _time

    sharded, in_names, out_names, out_avals, zero_outs, shard_spec = \
        _make_runner(nc, n_cores)
    concat_in = [
        np.concatenate([np.asarray(in_maps[c][n]) for c in range(n_cores)],
                       axis=0)
        for n in in_names
    ]
    staged_in = [jax.device_put(a, shard_spec) for a in concat_in]
    jax.block_until_ready(staged_in)

    def make_zeros():
        zs = [jax.device_put(
            np.zeros((n_cores * z.shape[0], *z.shape[1:]), z.dtype),
            shard_spec) for z in zero_outs]
        jax.block_until_ready(zs)
        return zs

    out_arrs = sharded(*staged_in, *make_zeros())
    jax.block_until_ready(out_arrs)

    times = []
    for _ in range(time_iters):
        zs = make_zeros()
        t0 = _time.perf_counter()
        out_arrs2 = sharded(*staged_in, *zs)
        jax.block_until_ready(out_arrs2)
        times.append(_time.perf_counter() - t0)

    results = [
        {name: np.asarray(out_arrs[i]).reshape(n_cores, *out_avals[i].shape)[c]
         for i, name in enumerate(out_names)}
        for c in range(n_cores)
    ]
    return results, (min(times) if times else None)


def postprocess(cfg, results):
    Nc = cfg.Nc

    def gather(name):
        return np.concatenate([results[c][name][:Nc] for c in range(NCORES)],
                              axis=0)

    return (gather("z_out"), gather("emb_out"), gather("ret_out"),
            gather("reta_out"))


def kernel(**inputs):
    cfg, in_maps = preprocess(inputs)
    nc = build_program(cfg)
    results, _ = _run(nc, in_maps, NCORES, time_iters=0)
    return postprocess(cfg, results)


def kernel_traced(time_iters=5, tab1_bf16=TAB1_BF16, tab2_bf16=TAB2_BF16,
                  **inputs):
    """Like kernel() but also times repeated executions (wall clock)."""
    cfg, in_maps = preprocess(inputs, tab1_bf16, tab2_bf16)
    nc = build_program(cfg)
    results, best = _run(nc, in_maps, NCORES, time_iters=time_iters)
    return postprocess(cfg, results), best


# revision 18
# speedup vs baseline: 8.6353x; 7.4405x over previous
"""Trainium2 Bass kernel for nn_Encoder_3796751090357 (GNN message passing).

Reference computation (see reference.py):
    x1   = feat   @ W1                      [N, 64]
    z    = S @ x1        (S = sparse adj)   [N, 64]   -> output "hidden_emb"
    emb  = relu(z)                                    -> output "emb"
    x2   = feat_a @ W1
    z_a  = S @ x2 ; emb_a = relu(z_a)
    g    = sigmoid(l2norm((S @ emb)   / rowsum(S)))
    g_a  = sigmoid(l2norm((S @ emb_a) / rowsum(S)))
    ret   = [bilin(emb, g),   bilin(emb_a, g)]        -> output [N, 2]
    ret_a = [bilin(emb_a, g_a), bilin(emb, g_a)]      -> output [N, 2]
    (the reference's `h` tensor is computed but unused -> skipped here)

Sharding: nodes (destination rows) are sharded across the 8 cores; edges are
partitioned by destination row.  Each core computes x1/x2 for its node shard;
an AllGather materializes the full [100352, 128] gather table in every core's
DRAM; each core then runs the SpMMs for its destination shard as one-hot
matmuls: for each 128-row destination block, edges are packed into chunks of
128; each chunk contributes onehot[e, r] = val_e * (lrow_e == r) as PE
weights against the dma_gather'ed source rows as the moving operand,
accumulating the block in PSUM.

The per-edge source rows are fetched with the GPSIMD dma_gather custom op
(mlp ucode library; int16 indices), so the table is split into 4 ranges of
2^15 rows; each destination block issues one gather per range.  Edge slots
are ordered (range, chunk, partition) to match dma_gather's output layout
(index k -> partition k%128, column k//128).
"""

from contextlib import ExitStack

import numpy as np
import ml_dtypes

import concourse.bacc as bacc
import concourse.bass as bass
import concourse.mybir as mybir
import concourse.tile as tile
from concourse.library_config import mlp as _mlp_lib
from concourse.tile import add_dep_helper
from concourse.masks import make_identity  # noqa: F401  (kept for reference)

F32 = mybir.dt.float32
BF16 = mybir.dt.bfloat16
I16 = mybir.dt.int16

NCORES = 8
BLK = 128
KDIM = 512  # IN_F
F = 64      # OUT_F
EPS = 1e-12
RANGE_BITS = 15  # int16 gather-index limit (2**15 rows per range)
RANGE_ROWS = 1 << RANGE_BITS

# gather-table dtypes: phase 1 (x1|x2 -> z) and phase 2 (emb|emb_a -> vsum)
TAB1_BF16 = False
TAB2_BF16 = False


class Cfg:
    def __init__(self, n_nodes, ct_r, tab1_bf16, tab2_bf16):
        self.RNG_ROWS = 0  # set by preprocess/build
        self.N = n_nodes
        assert n_nodes % NCORES == 0
        self.Nc = n_nodes // NCORES            # real rows per core
        self.NB = -(-self.Nc // BLK)           # dest blocks per core
        self.NPAD = self.NB * BLK              # padded rows per core
        self.NFULL = self.NPAD * NCORES        # padded rows, all cores
        self.CT_R = list(ct_r)                 # chunks per block, per range
        self.CT = sum(ct_r)                    # total chunks per block
        self.NR = len(ct_r)
        self.tab1_bf16 = tab1_bf16
        self.tab2_bf16 = tab2_bf16


def _npdt(bf16):
    return ml_dtypes.bfloat16 if bf16 else np.float32


# ----------------------------------------------------------------- host side

def preprocess(inputs, tab1_bf16=TAB1_BF16, tab2_bf16=TAB2_BF16):
    """Sort/pad edges, pre-transpose features, build per-core input maps."""
    feat = np.ascontiguousarray(np.asarray(inputs["feat"], dtype=np.float32))
    feat_a = np.ascontiguousarray(np.asarray(inputs["feat_a"], dtype=np.float32))
    vals = np.asarray(inputs["adj_vals"], dtype=np.float32)
    rows = np.asarray(inputs["adj_rows"]).astype(np.int64)
    cols = np.asarray(inputs["adj_cols"]).astype(np.int64)
    w1 = np.ascontiguousarray(np.asarray(inputs["weight1"], dtype=np.float32))
    disc_w = np.asarray(inputs["disc_w"], dtype=np.float32)
    disc_b = np.float32(np.asarray(inputs["disc_b"]))

    n_nodes = feat.shape[0]
    assert n_nodes % NCORES == 0
    nc_rows = n_nodes // NCORES
    nb = -(-nc_rows // BLK)
    npad = nb * BLK
    nfull = npad * NCORES
    n_ranges = -(-nfull // RANGE_ROWS)
    rng_rows = -(-nfull // n_ranges)  # equal-size ranges (<= RANGE_ROWS)

    # padded source-row id in the allgathered table + its range
    pid = (cols // nc_rows) * npad + cols % nc_rows
    rng_id = pid // rng_rows

    shard = rows // nc_rows
    lrow_core = rows - shard * nc_rows
    pblock = shard * nb + lrow_core // BLK        # global dest block id
    lr_all = (lrow_core % BLK).astype(np.float32)

    # order edges by (dest block, source range)
    key = pblock * n_ranges + rng_id
    order = np.argsort(key, kind="stable")
    key_s = key[order]
    pid_s = pid[order]
    vals_s = vals[order]
    lr_s = lr_all[order]

    n_groups = NCORES * nb * n_ranges
    counts = np.bincount(key_s, minlength=n_groups)
    cnt_br = counts.reshape(NCORES * nb, n_ranges)
    ct_r = [int(-(-cnt_br[:, r].max() // BLK)) for r in range(n_ranges)]
    ct_r = [max(c, 1) for c in ct_r]
    cfg = Cfg(n_nodes, ct_r, tab1_bf16, tab2_bf16)
    cfg.RNG_ROWS = rng_rows
    ct_tot = cfg.CT
    off_c = np.concatenate([[0], np.cumsum(ct_r)])  # chunk offset per range

    starts = np.zeros(n_groups, dtype=np.int64)
    np.cumsum(counts[:-1], out=starts[1:])
    rank = np.arange(len(key_s), dtype=np.int64) - starts[key_s]

    g_rng = key_s % n_ranges
    g_blk = key_s // n_ranges          # global block id (core*nb + b)
    ci = g_blk // nb
    bi = g_blk % nb

    # int16 gather indices: wrapped [16, S] layout, k -> (k%16, k//16);
    # per-range blocks concatenated along S. (S units of 16 idxs.)
    s_r = [c * BLK // 16 for c in ct_r]
    off_s = np.concatenate([[0], np.cumsum(s_r)])
    s_tot = int(off_s[-1])
    idx16 = np.full((NCORES, nb, 16, s_tot), -1, dtype=np.int16)
    idx16[ci, bi, rank % 16, off_s[g_rng] + rank // 16] = \
        (pid_s - g_rng * rng_rows).astype(np.int16)
    idx16 = np.ascontiguousarray(np.tile(idx16, (1, 1, 8, 1)))
    cnts = np.ascontiguousarray(
        cnt_br.reshape(NCORES, nb * n_ranges)[:, None, :].astype(np.int32))

    # one-hot operands: slot (p, chunk) with chunk = off_c[range] + rank//128
    lrowf = np.zeros((NCORES, nb, BLK, ct_tot), dtype=np.float32)
    valsf = np.zeros((NCORES, nb, BLK, ct_tot), dtype=np.float32)
    cslot = off_c[g_rng] + rank // BLK
    lrowf[ci, bi, rank % BLK, cslot] = lr_s
    valsf[ci, bi, rank % BLK, cslot] = vals_s

    # pre-transposed feature shards [KDIM, NPAD]
    fT = np.ascontiguousarray(feat.T)
    faT = np.ascontiguousarray(feat_a.T)
    k = feat.shape[1]

    wt2 = np.ascontiguousarray(
        np.concatenate([disc_w.T, disc_w.T], axis=0).astype(np.float32))
    bcol = np.full((BLK, 1), disc_b, dtype=np.float32)
    iota = np.broadcast_to(np.arange(BLK, dtype=np.float32)[None, :],
                           (BLK, BLK)).copy()
    ident = np.eye(BLK, dtype=np.float32)

    d1 = _npdt(tab1_bf16)
    d2 = _npdt(tab2_bf16)
    lv = np.ascontiguousarray(
        np.concatenate([lrowf, valsf], axis=3))  # [NC, nb, BLK, 2*CT]

    in_maps = []
    for c in range(NCORES):
        sl = slice(c * nc_rows, (c + 1) * nc_rows)
        ft = np.zeros((k, npad), dtype=np.float32)
        ft[:, :nc_rows] = fT[:, sl]
        fat = np.zeros((k, npad), dtype=np.float32)
        fat[:, :nc_rows] = faT[:, sl]
        in_maps.append({
            "featT": ft,
            "feataT": fat,
            "idx16": idx16[c],
            "lv": lv[c],
            "cnts": cnts[c],
            "w1": w1,
            "wt2": wt2,
            "bcol": bcol,
            "iota_in": iota,
            "ident_in": ident,
        })
    return cfg, in_maps


# --------------------------------------------------------------- device side

def build_program(cfg, kdim=KDIM, phases=(1, 1, 1), p2lvl=3):
    NB, CT, NPAD, NFULL = cfg.NB, cfg.CT, cfg.NPAD, cfg.NFULL
    CT_R, NR = cfg.CT_R, cfg.NR
    KC = kdim // BLK  # contraction chunks for the dense matmul
    DT1 = BF16 if cfg.tab1_bf16 else F32
    DT2 = BF16 if cfg.tab2_bf16 else F32
    RR = cfg.RNG_ROWS or RANGE_ROWS
    S_R = [c * BLK // 16 for c in CT_R]
    S_TOT = sum(S_R)
    OFF_S = np.concatenate([[0], np.cumsum(S_R)]).astype(int)
    OFF_C = np.concatenate([[0], np.cumsum(CT_R)]).astype(int)

    nc = bacc.Bacc("TRN2", target_bir_lowering=False, debug=False,
                   num_devices=NCORES)

    # kernel I/O
    featT = nc.dram_tensor("featT", [kdim, NPAD], F32, kind="ExternalInput").ap()
    feataT = nc.dram_tensor("feataT", [kdim, NPAD], F32, kind="ExternalInput").ap()
    idx16 = nc.dram_tensor("idx16", [NB, BLK, S_TOT], I16,
                           kind="ExternalInput").ap()
    lv = nc.dram_tensor("lv", [NB, BLK, 2 * CT], F32,
                        kind="ExternalInput").ap()
    cnts = nc.dram_tensor("cnts", [1, NB * NR], mybir.dt.int32,
                          kind="ExternalInput").ap()
    w1 = nc.dram_tensor("w1", [kdim, F], F32, kind="ExternalInput").ap()
    wt2 = nc.dram_tensor("wt2", [BLK, F], F32, kind="ExternalInput").ap()
    bcol = nc.dram_tensor("bcol", [BLK, 1], F32, kind="ExternalInput").ap()
    iota_in = nc.dram_tensor("iota_in", [BLK, BLK], F32,
                             kind="ExternalInput").ap()
    ident_in = nc.dram_tensor("ident_in", [BLK, BLK], F32,
                              kind="ExternalInput").ap()

    z_out = nc.dram_tensor("z_out", [NPAD, F], F32, kind="ExternalOutput").ap()
    emb_out = nc.dram_tensor("emb_out", [NPAD, F], F32, kind="ExternalOutput").ap()
    ret_out = nc.dram_tensor("ret_out", [NPAD, 2], F32, kind="ExternalOutput").ap()
    reta_out = nc.dram_tensor("reta_out", [NPAD, 2], F32,
                              kind="ExternalOutput").ap()

    with tile.TileContext(nc) as tc, ExitStack() as top:
        dram = top.enter_context(tc.tile_pool(name="dram", bufs=1, space="DRAM"))
        consts = top.enter_context(tc.tile_pool(name="consts", bufs=1))

        lib = nc.gpsimd.load_library(_mlp_lib)
        reg_ni = top.enter_context(nc.gpsimd.register("reg_ni"))

        def gather_rows(gat, idx_t, table, cnts_sb, b):
            """Per-range gathers of one destination block into `gat`."""
            for r in range(NR):
                ni = CT_R[r] * BLK
                o0 = int(OFF_C[r]) * 2 * F
                nc.gpsimd.reg_load(reg_ni,
                                   cnts_sb[0:1, b * NR + r:b * NR + r + 1])
                gi = nc.gpsimd.dma_gather(
                    out_ap=gat[:, o0:o0 + CT_R[r] * 2 * F]
                    .rearrange("p (c f) -> p c f", f=2 * F),
                    in_ap=table[r * RR:min((r + 1) * RR, NFULL), :],
                    idxs_ap=idx_t[:, OFF_S[r]:OFF_S[r] + S_R[r]],
                    num_idxs=ni, num_idxs_reg=reg_ni, elem_size=2 * F,
                    single_packet=False)
                add_dep_helper(gi.ins, lib.ins, reason="mlp lib before gather")

        # internal DRAM: collective bounce buffers + full gather tables
        t12_in = dram.tile([NPAD, 2 * F], DT1, name="t12_in")
        t12_full = dram.tile([NFULL, 2 * F], DT1, name="t12_full")
        temb_in = dram.tile([NPAD, 2 * F], DT2, name="temb_in")
        temb_full = dram.tile([NFULL, 2 * F], DT2, name="temb_full")
        rs_dram = dram.tile([NPAD, 1], F32, name="rs_dram")
        # fp32 staging of emb|emb_a for the bilinear reload when DT2 is bf16
        embf = temb_in if DT2 == F32 else dram.tile([NPAD, 2 * F], F32,
                                                    name="embf")

        # constants
        ident = consts.tile([BLK, BLK], F32, name="ident")
        nc.sync.dma_start(out=ident[:], in_=ident_in)
        iota_1 = consts.tile([BLK, BLK], DT1, name="iota_1")
        nc.sync.dma_start(out=iota_1[:], in_=iota_in) if DT1 == F32 else None
        iota_f = None
        if DT1 != F32 or DT2 != F32:
            iota_f = consts.tile([BLK, BLK], F32, name="iota_f")
            nc.sync.dma_start(out=iota_f[:], in_=iota_in)
        if DT1 != F32:
            nc.vector.tensor_copy(iota_1[:], iota_f[:])
        if DT2 == DT1:
            iota_2 = iota_1
        else:
            iota_2 = consts.tile([BLK, BLK], DT2, name="iota_2")
            if DT2 == F32:
                nc.sync.dma_start(out=iota_2[:], in_=iota_in)
            else:
                nc.vector.tensor_copy(iota_2[:], iota_f[:])
        ones1 = consts.tile([BLK, 1], DT1, name="ones1")
        nc.vector.memset(ones1[:], 1.0)
        w1_sb = consts.tile([BLK, KC, F], F32, name="w1_sb")
        nc.sync.dma_start(out=w1_sb[:], in_=w1.rearrange("(k p) f -> p k f",
                                                         p=BLK))
        wt2_sb = consts.tile([BLK, F], F32, name="wt2_sb")
        nc.sync.dma_start(out=wt2_sb[:], in_=wt2)
        bcol_sb = consts.tile([BLK, 1], F32, name="bcol_sb")
        nc.sync.dma_start(out=bcol_sb[:], in_=bcol)
        cnts_sb = consts.tile([1, NB * NR], mybir.dt.int32, name="cnts_sb")
        nc.sync.dma_start(out=cnts_sb[:], in_=cnts)

        # ---------------- phase 0: x = feat @ W1 (per shard), both tables
        SBW = 1024  # n-columns of featT loaded per DMA
        with tc.tile_pool(name="p0", bufs=3) as p0, \
             tc.tile_pool(name="p0ps", bufs=4, space="PSUM") as p0ps:
            for t_i, src in enumerate((featT, feataT)):
                for s0 in range(0, NPAD, SBW):
                    sw = min(SBW, NPAD - s0)
                    fts = []
                    for kc in range(KC):
                        ft = p0.tile([BLK, sw], F32, name=f"ft{kc}",
                                     tag=f"ft{kc}", padded_shape=[BLK, SBW])
                        nc.sync.dma_start(
                            out=ft[:],
                            in_=src[kc * BLK:(kc + 1) * BLK, s0:s0 + sw])
                        fts.append(ft)
                    for nb0 in range(0, sw, BLK):
                        ps = p0ps.tile([BLK, F], F32, name="ps", tag="ps")
                        for kc in range(KC):
                            nc.tensor.matmul(
                                ps[:], lhsT=fts[kc][:, nb0:nb0 + BLK],
                                rhs=w1_sb[:, kc, :],
                                start=(kc == 0), stop=(kc == KC - 1))
                        xt = p0.tile([BLK, F], DT1, name="xt", tag="xt")
                        nc.vector.tensor_copy(xt[:], ps[:])
                        r0 = s0 + nb0
                        nc.sync.dma_start(
                            out=t12_in[r0:r0 + BLK, t_i * F:(t_i + 1) * F],
                            in_=xt[:])

        nc.gpsimd.collective_compute(
            "AllGather", mybir.AluOpType.bypass,
            replica_groups=[list(range(NCORES))],
            ins=[t12_in.opt()], outs=[t12_full.opt()])

        # ---------------- phase 1: z|z_a = S @ (x1|x2), rowsum
        with tc.tile_pool(name="p1", bufs=3) as p1, \
             tc.tile_pool(name="p1g", bufs=2) as p1g, \
             tc.tile_pool(name="p1ps", bufs=3, space="PSUM") as p1ps:
            NG = 3
            g1ts = []
            for i in range(NB and NG):
                g1 = p1g.tile([BLK, CT * 2 * F], DT1, name=f"g1_{i}",
                              tag=f"g1_{i}", bufs=1)
                nc.vector.memset(g1[:], 0.0)
                g1ts.append(g1)
            for b in range(NB if phases[1] else 0):
                idx_t = p1.tile([BLK, S_TOT], I16, name="idx_t", tag="idx")
                nc.sync.dma_start(out=idx_t[:], in_=idx16[b])
                lv_t = p1.tile([BLK, 2 * CT], F32, name="lv_t", tag="lv")
                nc.sync.dma_start(out=lv_t[:], in_=lv[b])

                gat = g1ts[b % NG]
                gather_rows(gat, idx_t, t12_full, cnts_sb, b)

                oh = p1g.tile([BLK, CT * BLK], DT1, name="oh", tag="oh")
                for j in range(CT):
                    nc.vector.tensor_scalar(
                        out=oh[:, j * BLK:(j + 1) * BLK], in0=iota_1[:],
                        scalar1=lv_t[:, j:j + 1],
                        scalar2=lv_t[:, CT + j:CT + j + 1],
                        op0=mybir.AluOpType.is_equal,
                        op1=mybir.AluOpType.mult)

                ps = p1ps.tile([BLK, 2 * F], F32, name="psz", tag="psz")
                ps_rs = p1ps.tile([BLK, 1], F32, name="psrs", tag="psrs")
                for j in range(CT):
                    nc.tensor.matmul(
                        ps[:], lhsT=oh[:, j * BLK:(j + 1) * BLK],
                        rhs=gat[:, j * 2 * F:(j + 1) * 2 * F],
                        start=(j == 0), stop=(j == CT - 1))
                    nc.tensor.matmul(
                        ps_rs[:], lhsT=oh[:, j * BLK:(j + 1) * BLK],
                        rhs=ones1[:],
                        start=(j == 0), stop=(j == CT - 1))

                r0 = b * BLK
                zt = p1.tile([BLK, F], F32, name="zt", tag="zt")
                nc.vector.tensor_copy(zt[:], ps[:, 0:F])
                nc.scalar.dma_start(out=z_out[r0:r0 + BLK, :], in_=zt[:])
                rst = p1.tile([BLK, 1], F32, name="rst", tag="rst")
                nc.vector.tensor_copy(rst[:], ps_rs[:])
                nc.scalar.dma_start(out=rs_dram[r0:r0 + BLK, :], in_=rst[:])
                embt = p1.tile([BLK, 2 * F], DT2, name="embt", tag="embt")
                nc.scalar.activation(embt[:], ps[:],
                                     mybir.ActivationFunctionType.Relu)
                nc.scalar.dma_start(out=temb_in[r0:r0 + BLK, :], in_=embt[:])
                if DT2 == F32:
                    nc.scalar.dma_start(out=emb_out[r0:r0 + BLK, :],
                                        in_=embt[:, 0:F])
                else:
                    embtf = p1.tile([BLK, 2 * F], F32, name="embtf",
                                    tag="embtf")
                    nc.scalar.activation(embtf[:], ps[:],
                                         mybir.ActivationFunctionType.Relu)
                    nc.scalar.dma_start(out=embf[r0:r0 + BLK, :], in_=embtf[:])
                    nc.scalar.dma_start(out=emb_out[r0:r0 + BLK, :],
                                        in_=embtf[:, 0:F])

        nc.gpsimd.collective_compute(
            "AllGather", mybir.AluOpType.bypass,
            replica_groups=[list(range(NCORES))],
            ins=[temb_in.opt()], outs=[temb_full.opt()])

        # ---------------- phase 2: vsum | vsum_a, readout, bilinear
        with tc.tile_pool(name="p2", bufs=3) as p2, \
             tc.tile_pool(name="p2g", bufs=2) as p2g, \
             tc.tile_pool(name="p2ps", bufs=4, space="PSUM") as p2ps, \
             tc.tile_pool(name="p2bp", bufs=1, space="PSUM") as p2bp:
            NG = 3
            g2ts = []
            for i in range(NB and NG):
                g2 = p2g.tile([BLK, CT * 2 * F], DT2, name=f"g2_{i}",
                              tag=f"g2_{i}", bufs=1)
                nc.vector.memset(g2[:], 0.0)
                g2ts.append(g2)
            for b in range(NB if phases[2] else 0):
                idx_t = p2.tile([BLK, S_TOT], I16, name="idx_t2", tag="idx")
                nc.sync.dma_start(out=idx_t[:], in_=idx16[b])
                lv_t = p2.tile([BLK, 2 * CT], F32, name="lv_t2", tag="lv")
                nc.sync.dma_start(out=lv_t[:], in_=lv[b])

                gat = g2ts[b % NG]
                gather_rows(gat, idx_t, temb_full, cnts_sb, b)

                oh = p2g.tile([BLK, CT * BLK], DT2, name="oh2", tag="oh")
                for j in range(CT):
                    nc.vector.tensor_scalar(
                        out=oh[:, j * BLK:(j + 1) * BLK], in0=iota_2[:],
                        scalar1=lv_t[:, j:j + 1],
                        scalar2=lv_t[:, CT + j:CT + j + 1],
                        op0=mybir.AluOpType.is_equal,
                        op1=mybir.AluOpType.mult)

                ps = p2ps.tile([BLK, 2 * F], F32, name="psv", tag="psv")
                for j in range(CT):
                    nc.tensor.matmul(
                        ps[:], lhsT=oh[:, j * BLK:(j + 1) * BLK],
                        rhs=gat[:, j * 2 * F:(j + 1) * 2 * F],
                        start=(j == 0), stop=(j == CT - 1))

                r0 = b * BLK
                if p2lvl == 1:
                    jt = p2.tile([BLK, 2], F32, name="jt", tag="rett")
                    nc.vector.tensor_copy(jt[:], ps[:, 0:2])
                    nc.sync.dma_start(out=ret_out[r0:r0 + BLK, :], in_=jt[:])
                    continue
                # readout: g = sigmoid(l2norm(vsum / rowsum))
                rsl = p2.tile([BLK, 1], F32, name="rsl", tag="rsl")
                nc.sync.dma_start(out=rsl[:], in_=rs_dram[r0:r0 + BLK, :])
                rsx = p2.tile([BLK, 1], F32, name="rsx", tag="rsx")
                nc.vector.tensor_scalar_max(rsx[:], rsl[:], 1e-30)
                inv = p2.tile([BLK, 1], F32, name="inv", tag="inv")
                nc.vector.reciprocal(inv[:], rsx[:])
                gv = p2.tile([BLK, 2 * F], F32, name="gv", tag="gv")
                nc.vector.tensor_scalar_mul(gv[:], ps[:], inv[:])
                sq = p2.tile([BLK, 2 * F], F32, name="sq", tag="sq")
                nc.vector.tensor_tensor(sq[:], gv[:], gv[:],
                                        op=mybir.AluOpType.mult)
                ss = p2.tile([BLK, 2], F32, name="ss", tag="ss")
                nc.vector.tensor_reduce(
                    ss[:], sq.rearrange("p (t f) -> p t f", f=F),
                    axis=mybir.AxisListType.X, op=mybir.AluOpType.add)
                nrm = p2.tile([BLK, 2], F32, name="nrm", tag="nrm")
                nc.scalar.activation(nrm[:], ss[:],
                                     mybir.ActivationFunctionType.Sqrt)
                nc.vector.tensor_scalar_max(nrm[:], nrm[:], EPS)
                rinv = p2.tile([BLK, 2], F32, name="rinv", tag="rinv")
                nc.vector.reciprocal(rinv[:], nrm[:])
                gb = p2.tile([BLK, 2 * F], F32, name="gb", tag="gb")
                nc.scalar.activation(gb[:, 0:F], gv[:, 0:F],
                                     mybir.ActivationFunctionType.Sigmoid,
                                     scale=rinv[:, 0:1])
                nc.scalar.activation(gb[:, F:2 * F], gv[:, F:2 * F],
                                     mybir.ActivationFunctionType.Sigmoid,
                                     scale=rinv[:, 1:2])

                if p2lvl == 2:
                    jt = p2.tile([BLK, 2], F32, name="jt2", tag="rett")
                    nc.vector.tensor_copy(jt[:], gb[:, 0:2])
                    nc.sync.dma_start(out=ret_out[r0:r0 + BLK, :], in_=jt[:])
                    continue
                # bilinear: Wg = disc_w @ g[n], ret = rowsum(emb * Wg) + b
                tp1 = p2bp.tile([BLK, BLK], F32, name="tp1", tag="tp1")
                nc.tensor.transpose(tp1[:], gb[:], ident[:])
                gT = p2.tile([BLK, BLK], F32, name="gT", tag="gT")
                nc.vector.tensor_copy(gT[:], tp1[:])
                wgp = p2bp.tile([BLK, BLK], F32, name="wgp", tag="wgp")
                nc.tensor.matmul(wgp[0:F, :], lhsT=wt2_sb[0:F, :],
                                 rhs=gT[0:F, :], start=True, stop=True)
                nc.tensor.matmul(wgp[F:BLK, :], lhsT=wt2_sb[F:BLK, :],
                                 rhs=gT[F:BLK, :], start=True, stop=True)
                if p2lvl == 21:
                    jt = p2.tile([BLK, 2], F32, name="jt3", tag="rett")
                    nc.vector.tensor_copy(jt[:], gT[:, 0:2])
                    nc.sync.dma_start(out=ret_out[r0:r0 + BLK, :], in_=jt[:])
                    continue
                wgT = p2.tile([BLK, BLK], F32, name="wgT", tag="wgT")
                nc.vector.tensor_copy(wgT[:], wgp[:])
                tp2 = p2bp.tile([BLK, BLK], F32, name="tp2", tag="tp2")
                nc.tensor.transpose(tp2[:], wgT[:], ident[:])
                if p2lvl == 22:
                    jt = p2.tile([BLK, 2], F32, name="jt4", tag="rett")
                    nc.vector.tensor_copy(jt[:], wgT[:, 0:2])
                    nc.sync.dma_start(out=ret_out[r0:r0 + BLK, :], in_=jt[:])
                    continue
                wg = p2.tile([BLK, BLK], F32, name="wg", tag="wg")
                nc.vector.tensor_copy(wg[:], tp2[:])

                embt = p2.tile([BLK, 2 * F], F32, name="embt2", tag="embt")
                nc.sync.dma_start(out=embt[:], in_=embf[r0:r0 + BLK, :])

                rett = p2.tile([BLK, 2], F32, name="rett", tag="rett")
                retat = p2.tile([BLK, 2], F32, name="retat", tag="retat")
                prod = p2.tile([BLK, 2 * F], F32, name="prod", tag="prod")
                prod3 = prod.rearrange("p (t f) -> p t f", f=F)
                # red[t] = rowsum(embt_half_t * Wg_half): (emb, emb_a) order
                ein = embt.rearrange("p (t f) -> p t f", f=F)
                for dst, w_sl, swap in ((rett, slice(0, F), False),
                                        (retat, slice(F, 2 * F), True)):
                    win = wg[:, w_sl].unsqueeze(1).broadcast_to([BLK, 2, F])
                    nc.vector.tensor_tensor(out=prod3, in0=ein, in1=win,
                                            op=mybir.AluOpType.mult)
                    red = p2.tile([BLK, 2], F32, name="red", tag="red")
                    nc.vector.tensor_reduce(red[:], prod3,
                                            axis=mybir.AxisListType.X,
                                            op=mybir.AluOpType.add)
                    # ret=(emb.Wg, emb_a.Wg); ret_a=(emb_a.Wg_a, emb.Wg_a)
                    for di, ri in ((0, 1), (1, 0)) if swap else ((0, 0),
                                                                (1, 1)):
                        nc.vector.tensor_scalar(
                            out=dst[:, di:di + 1], in0=red[:, ri:ri + 1],
                            scalar1=bcol_sb[:], scalar2=None,
                            op0=mybir.AluOpType.add)
                nc.scalar.dma_start(out=ret_out[r0:r0 + BLK, :], in_=rett[:])
                nc.scalar.dma_start(out=reta_out[r0:r0 + BLK, :], in_=retat[:])

    nc.compile()
    return nc


# ------------------------------------------------------------------- running

def _make_runner(nc, n_cores):
    """Replicates bass2jax.run_bass_via_pjrt's jit construction, returning a
    callable we can invoke repeatedly (for timing) with pre-staged inputs."""
    import jax
    from jax.sharding import Mesh, PartitionSpec, NamedSharding
    from jax.experimental.shard_map import shard_map
    from concourse import bass2jax

    bass2jax.install_neuronx_cc_hook()

    partition_name = (nc.partition_id_tensor.name
                      if nc.partition_id_tensor else None)
    in_names, out_names, out_avals, zero_outs = [], [], [], []
    for alloc in nc.m.functions[0].allocations:
        if not isinstance(alloc, mybir.MemoryLocationSet):
            continue
        name = alloc.memorylocations[0].name
        if alloc.kind == "ExternalInput":
            if name != partition_name:
                in_names.append(name)
        elif alloc.kind == "ExternalOutput":
            shape = tuple(alloc.tensor_shape)
            dtype = mybir.dt.np(alloc.dtype)
            out_names.append(name)
            out_avals.append(jax.core.ShapedArray(shape, dtype))
            zero_outs.append(np.zeros(shape, dtype))
    n_params = len(in_names)
    n_outs = len(out_avals)
    in_names = in_names + out_names
    if partition_name is not None:
        in_names.append(partition_name)
    donate = tuple(range(n_params, n_params + n_outs))

    def _body(*args):
        operands = list(args)
        if partition_name is not None:
            operands.append(bass2jax.partition_id_tensor())
        outs = bass2jax._bass_exec_p.bind(
            *operands,
            out_avals=tuple(out_avals),
            in_names=tuple(in_names),
            out_names=tuple(out_names),
            lowering_input_output_aliases=(),
            sim_require_finite=True,
            sim_require_nnan=True,
            nc=nc,
        )
        return tuple(outs)

    devices = jax.devices()[:n_cores]
    mesh = Mesh(np.asarray(devices), ("core",))
    sharded = jax.jit(
        shard_map(_body, mesh=mesh,
                  in_specs=(PartitionSpec("core"),) * (n_params + n_outs),
                  out_specs=(PartitionSpec("core"),) * n_outs,
                  check_rep=False),
        donate_argnums=donate, keep_unused=True)
    shard_spec = NamedSharding(mesh, PartitionSpec("core"))
    return sharded, in_names[:n_params], out_names, out_avals, zero_outs, \
        shard_spec


def _run(nc, in_maps, n_cores, time_iters=0):
    import jax
    import time as _time

    sharded, in_names, out_names, out_avals, zero_outs, shard_spec = \
        _make_runner(nc, n_cores)
    concat_in = [
        np.concatenate([np.asarray(in_maps[c][n]) for c in range(n_cores)],
                       axis=0)
        for n in in_names
    ]
    staged_in = [jax.device_put(a, shard_spec) for a in concat_in]
    jax.block_until_ready(staged_in)

    def make_zeros():
        zs = [jax.device_put(
            np.zeros((n_cores * z.shape[0], *z.shape[1:]), z.dtype),
            shard_spec) for z in zero_outs]
        jax.block_until_ready(zs)
        return zs

    out_arrs = sharded(*staged_in, *make_zeros())
    jax.block_until_ready(out_arrs)

    times = []
    for _ in range(time_iters):
        zs = make_zeros()
        t0 = _time.perf_counter()
        out_arrs2 = sharded(*staged_in, *zs)
        jax.block_until_ready(out_arrs2)
        times.append(_time.perf_counter() - t0)

    results = [
        {name: np.asarray(out_arrs[i]).reshape(n_cores, *out_avals[i].shape)[c]
         for i, name in enumerate(out_names)}
        for c in range(n_cores)
    ]
    return results, (min(times) if times else None)


def time_pipelined(nc, in_maps, n_cores, n_calls=10):
    """Issue n_calls back-to-back executions without blocking in between;
    the marginal per-call time approximates device exec + marginal RPC."""
    import jax
    import time as _time

    sharded, in_names, out_names, out_avals, zero_outs, shard_spec = \
        _make_runner(nc, n_cores)
    concat_in = [
        np.concatenate([np.asarray(in_maps[c][n]) for c in range(n_cores)],
                       axis=0)
        for n in in_names
    ]
    staged_in = [jax.device_put(a, shard_spec) for a in concat_in]
    jax.block_until_ready(staged_in)
    zs_all = []
    for _ in range(n_calls + 1):
        zs_all.append([jax.device_put(
            np.zeros((n_cores * z.shape[0], *z.shape[1:]), z.dtype),
            shard_spec) for z in zero_outs])
    jax.block_until_ready(zs_all)
    # warm up
    jax.block_until_ready(sharded(*staged_in, *zs_all[0]))
    t0 = _time.perf_counter()
    outs = [sharded(*staged_in, *zs_all[i + 1]) for i in range(n_calls)]
    jax.block_until_ready(outs)
    t_all = _time.perf_counter() - t0
    return t_all / n_calls


def postprocess(cfg, results):
    Nc = cfg.Nc

    def gather(name):
        return np.concatenate([results[c][name][:Nc] for c in range(NCORES)],
                              axis=0)

    return (gather("z_out"), gather("emb_out"), gather("ret_out"),
            gather("reta_out"))


def kernel(**inputs):
    cfg, in_maps = preprocess(inputs)
    nc = build_program(cfg)
    results, _ = _run(nc, in_maps, NCORES, time_iters=0)
    return postprocess(cfg, results)


def kernel_traced(time_iters=5, tab1_bf16=TAB1_BF16, tab2_bf16=TAB2_BF16,
                  **inputs):
    """Like kernel() but also times repeated executions (wall clock)."""
    cfg, in_maps = preprocess(inputs, tab1_bf16, tab2_bf16)
    nc = build_program(cfg)
    results, best = _run(nc, in_maps, NCORES, time_iters=time_iters)
    return postprocess(cfg, results), best
